# revision 3
# baseline (speedup 1.0000x reference)
"""GATv2 x3 + MLP (nn_GAT) on trn2, 8 NeuronCores.

v2 launch1: stationary-swapped projection (edge-stream is the matmul
stationary operand, block-diag weights are the moving operand) writes
node-major bf16 tiles straight into PSUM -- no PE transposes, no
PSUM->SBUF copy pass.  Softmax scores use the lrelu decomposition
lrelu(v) = 0.6 v + 0.4|v|: the linear term rides along the projection as
two extra columns, the |.| term is two abs-reduces over sign-grouped
columns.  Per-edge exp() columns share the E tile so one tree reduction
over slots produces both the weighted sums and the softmax denominators.
Launch2 (d2 GAT + MLP) still the v1 transpose pipeline.
"""
import sys
sys.path.insert(0, '/opt/trn_rl_repo')
import numpy as np
import ml_dtypes

import concourse.bass as bass
import concourse.mybir as mybir
from concourse import bacc
from concourse.tile import TileContext
from concourse.bass_utils import run_bass_kernel_spmd
from concourse.masks import make_identity

bf16 = mybir.dt.bfloat16
f32 = mybir.dt.float32
BF = ml_dtypes.bfloat16
AL = mybir.AluOpType
AF = mybir.ActivationFunctionType
AX = mybir.AxisListType

NCORE = 8
P = 128
B = 8
NEG_GAT = 0.2
NEG_MLP = 0.01


# ================================================================= host prep
def build_schedule(dst, n):
    nloc = n // NCORE
    core_of = dst // nloc
    scheds = []
    for c in range(NCORE):
        em = np.where(core_of == c)[0]
        ldst = dst[em] - c * nloc
        deg = np.bincount(ldst, minlength=nloc)
        nt = -(-nloc // P)
        nt = -(-nt // B) * B
        degp = np.concatenate([deg, np.zeros(nt * P - nloc, np.int64)])
        order = np.argsort(-degp, kind='stable')
        pos_of = np.empty_like(order)
        pos_of[order] = np.arange(len(order))
        scheds.append(dict(core=c, em=em, ldst=ldst, deg=degp, order=order,
                           pos_of=pos_of, nt=nt, nloc=nloc))
    nt = scheds[0]['nt']
    nst = nt // B
    Ls = []
    for st in range(nst):
        L = 1
        for s in scheds:
            L = max(L, int(s['deg'][s['order'][st * B * P]]))
        Ls.append(L)
    # old (launch2) stream offsets
    offs = np.concatenate([[0], np.cumsum([B * L * 16 for L in Ls])]).astype(np.int64)
    # v2 (launch1) stream: Lp multiple of 8, [96, (Lp/2)*128] blocks
    Lps = [-(-L // 8) * 8 for L in Ls]
    offs2 = np.concatenate([[0], np.cumsum([(Lp // 2) * P for Lp in Lps])]).astype(np.int64)
    return scheds, nst, Ls, offs, Lps, offs2


def edge_pos(s):
    """Common per-edge placement: sorted-position, rank within node."""
    pos_e = s['pos_of'][s['ldst']]
    eo = np.lexsort((np.arange(len(pos_e)), pos_e))
    pos_sorted = pos_e[eo]
    starts = np.concatenate([[0], np.cumsum(s['deg'][s['order']])])
    rank = np.arange(len(eo)) - starts[pos_sorted]
    return eo, pos_sorted, rank


def edge_slot_cols_old(s, Ls, offs, eo, pos_sorted, rank):
    """v1 layout for launch2: q = b*L + l, 16-col groups, 8-row interleave."""
    st_of = (pos_sorted // P) // B
    L_e = np.asarray(Ls)[st_of]
    q_e = ((pos_sorted // P) % B) * L_e + rank
    col_e = (offs[st_of] + (q_e // 8) * P + (pos_sorted % P)).astype(np.int64)
    a_e = (q_e % 8).astype(np.int64)
    npad = np.zeros((P, s['nt']), np.float32)
    for t in range(s['nt']):
        L = Ls[t // B]
        npad[:, t] = L - s['deg'][s['order'][t * P:(t + 1) * P]]
    return a_e, col_e, npad


def edge_cols_v2(s, offs2, eo, pos_sorted, rank):
    """v2 layout for launch1: col = offs2[st] + (l//2)*128 + p,
    row-base = (l%2)*48 + b*6."""
    st_of = (pos_sorted // P) // B
    b_e = (pos_sorted // P) % B
    p_e = pos_sorted % P
    col_e = (offs2[st_of] + (rank // 2) * P + p_e).astype(np.int64)
    rb_e = ((rank % 2) * 48 + b_e * 6).astype(np.int64)
    return rb_e, col_e


def pack_edges_old(feats, eo, a_e, col_e, totc, nrow):
    pk = np.zeros((8 * nrow, totc), BF)
    fe = feats[eo].astype(BF)
    for f in range(nrow - 1):
        pk[a_e * nrow + f, col_e] = fe[:, f]
    pk[a_e * nrow + (nrow - 1), col_e] = BF(1.0)
    return pk


def pack_edges_v2(feats, eo, rb_e, col_e, totc2):
    pk = np.zeros((96, totc2), BF)
    fe = feats[eo].astype(BF)
    for f in range(5):
        pk[rb_e + f, col_e] = fe[:, f]
    pk[rb_e + 5, col_e] = BF(1.0)
    return pk


def pack_local(vals, nrow, nt):
    pk = np.zeros((8 * nrow, (nt // 8) * P), BF)
    nodes = np.arange(nt * P)
    a = (nodes // P) % 8
    col = (nodes // (8 * P)) * P + nodes % P
    v = vals.astype(BF)
    for f in range(nrow - 1):
        pk[a * nrow + f, col] = v[:, f]
    pk[a * nrow + (nrow - 1), col] = BF(1.0)
    return pk


def pack_xl16(vals, valid, nt, ngl):
    """[96, ngl*128]: tile t -> block t%16 rows (t%16)*6+f, cols (t//16)*128+p."""
    pk = np.zeros((96, ngl * P), BF)
    nodes = np.arange(nt * P)
    t = nodes // P
    p = nodes % P
    row_b = (t % 16) * 6
    col = (t // 16) * P + p
    v = vals.astype(BF)
    for f in range(5):
        pk[row_b + f, col] = v[:, f]
    pk[row_b + 5, col] = np.where(valid, BF(1.0), BF(0.0))
    return pk


def blockdiag(w, bias, nrow, sp=16):
    bd = np.zeros((8 * nrow, 8 * sp), np.float32)
    k = w.shape[1]
    for a in range(8):
        bd[a * nrow:a * nrow + w.shape[0], a * sp:a * sp + k] = w
        bd[a * nrow + nrow - 1, a * sp:a * sp + k] = bias
    return bd.astype(BF)


def pm(vals, nt):
    d = vals.shape[1]
    return np.ascontiguousarray(
        vals.reshape(nt, P, d).transpose(1, 0, 2).reshape(P, nt * d))


# ======================================================== v2 weight builders
def build_l1_weights(inp):
    """Column map c: 0,1 a1(|a|-folded); 2+5h+j d1 (h, perm_h[j]) |a|-folded;
    12,13 sigma-tilde(h); 14 VALID; 15 zero. Returns M6src, Mdst, Mres, metadata."""
    aW = np.asarray(inp['a1_Wsrc'], np.float64)
    ab = np.asarray(inp['a1_bsrc'], np.float64)
    aWd = np.asarray(inp['a1_Wdst'], np.float64)
    abd = np.asarray(inp['a1_bdst'], np.float64)
    aat = np.asarray(inp['a1_attn'], np.float64)[:, 0]      # [2]
    aWr = np.asarray(inp['a1_Wres'], np.float64)
    abr = np.asarray(inp['a1_bias'], np.float64)
    dW = np.asarray(inp['d1_Wsrc'], np.float64)
    db = np.asarray(inp['d1_bsrc'], np.float64)
    dWd = np.asarray(inp['d1_Wdst'], np.float64)
    dbd = np.asarray(inp['d1_bdst'], np.float64)
    dat = np.asarray(inp['d1_attn'], np.float64)            # [2,5]
    dWr = np.asarray(inp['d1_Wres'], np.float64)
    dbr = np.asarray(inp['d1_bias'], np.float64)

    sa = np.sign(aat)
    sa[sa == 0] = 1.0
    aa = np.abs(aat)
    aa[aa == 0] = 1e-12
    perms, nposs = [], []
    for h in range(2):
        fpos = [f for f in range(5) if dat[h, f] > 0]
        fneg = [f for f in range(5) if dat[h, f] <= 0]
        perms.append(fpos + fneg)
        nposs.append(len(fpos))
    ad = np.abs(dat)
    ad[ad == 0] = 1e-12

    def mk(W, bvec, Wa, ba, sig):
        """[6,16] src- or dst-side column matrix (sig: include sigma cols)."""
        M = np.zeros((6, 16), np.float64)
        for h in range(2):
            M[0:5, h] = aa[h] * Wa[:, h]
            M[5, h] = aa[h] * ba[h]
            for j, f in enumerate(perms[h]):
                M[0:5, 2 + 5 * h + j] = ad[h, f] * W[:, 5 * h + f]
                M[5, 2 + 5 * h + j] = ad[h, f] * bvec[5 * h + f]
            if sig:
                for f in range(5):
                    M[0:5, 12 + h] += dat[h, f] * W[:, 5 * h + f]
                    M[5, 12 + h] += dat[h, f] * bvec[5 * h + f]
        return M

    M6src = mk(dW, db, aW, ab, True)
    M6src[5, 14] = 1.0          # VALID indicator from the ones-row
    Mdst = mk(dWd, dbd, aWd, abd, True)
    Mres = np.zeros((6, 12), np.float64)
    for h in range(2):
        Mres[0:5, h] = aWr[:, h]
        Mres[5, h] = abr[h]
        for j, f in enumerate(perms[h]):
            Mres[0:5, 2 + 5 * h + j] = dWr[:, 5 * h + f]
            Mres[5, 2 + 5 * h + j] = dbr[5 * h + f]

    inva = np.zeros(12, np.float32)
    inva[0] = 1.0 / aa[0]
    inva[1] = 1.0 / aa[1]
    for h in range(2):
        for j, f in enumerate(perms[h]):
            inva[2 + 5 * h + j] = 1.0 / ad[h, f]

    # moving operand [96, 256]: row (lp*48 + b*6 + f), col (lp*128 + b*16 + c)
    bd2 = np.zeros((96, 256), np.float32)
    for lp in range(2):
        for b in range(8):
            bd2[lp * 48 + b * 6: lp * 48 + b * 6 + 6,
                lp * 128 + b * 16: lp * 128 + b * 16 + 16] = M6src
    # local projection moving operand [96, 448]: 16 blocks of [6, 28]
    Mloc = np.concatenate([Mdst, Mres], axis=1)  # [6, 28]
    ml16 = np.zeros((96, 448), np.float32)
    for tt in range(16):
        ml16[tt * 6: tt * 6 + 6, tt * 28: tt * 28 + 28] = Mloc
    return (bd2.astype(BF), ml16.astype(BF), inva,
            tuple(nposs), (float(sa[0]), float(sa[1])), perms)


# ============================================================ launch1 v2
def build_launch1_v2(nst, Lps, offs2, nt, ngl, npos, sa):
    totc2 = int(offs2[-1])
    nc = bacc.Bacc("TRN2", target_bir_lowering=False, debug=False, num_devices=NCORE)
    d_pk = nc.dram_tensor("x5e", [96, totc2], bf16, kind="ExternalInput")
    d_xl = nc.dram_tensor("x5l", [96, ngl * P], bf16, kind="ExternalInput")
    d_bd = nc.dram_tensor("bd2", [96, 256], bf16, kind="ExternalInput")
    d_ml = nc.dram_tensor("ml16", [96, 448], bf16, kind="ExternalInput")
    d_inva = nc.dram_tensor("inva", [P, 12], f32, kind="ExternalInput")
    d_h1o = nc.dram_tensor("h1o", [P, nt * 12], f32, kind="ExternalOutput")
    with TileContext(nc) as tc, nc.allow_low_precision("bf16 GAT partials"):
        with tc.tile_pool(name="res", bufs=1) as res, \
             tc.tile_pool(name="sb", bufs=2) as sb, \
             tc.tile_pool(name="sb3", bufs=3) as sb3, \
             tc.tile_pool(name="ps", bufs=2, space="PSUM") as ps, \
             tc.tile_pool(name="psl", bufs=2, space="PSUM") as psl:
            bd = res.tile([96, 256], bf16)
            nc.sync.dma_start(out=bd[:], in_=d_bd[:, :])
            ml = res.tile([96, 448], bf16)
            nc.sync.dma_start(out=ml[:], in_=d_ml[:, :])
            invat = res.tile([P, 12], f32)
            nc.sync.dma_start(out=invat[:], in_=d_inva[:, :])
            ftab = res.tile([P, nt * 16], bf16)
            ftabr = res.tile([P, nt * 12], bf16)
            h1o = res.tile([P, nt * 12], f32)
            # ---- local (dst/res) projections: 16 tiles per matmul
            for g in range(ngl):
                xls = sb.tile([96, P], bf16, tag="xls")
                nc.sync.dma_start(out=xls[:], in_=d_xl[:, g * P:(g + 1) * P])
                pl = psl.tile([P, 448], f32, tag="pl")
                nc.tensor.matmul(out=pl[:], lhsT=xls[:], rhs=ml[:],
                                 start=True, stop=True)
                pl3 = pl[:].rearrange("p (t c) -> p t c", t=16, c=28)
                ntile = min(16, nt - g * 16)
                fslice = ftab[:].rearrange("p (t c) -> p t c", t=nt, c=16)[
                    :, g * 16:g * 16 + ntile, :]
                nc.scalar.copy(out=fslice, in_=pl3[:, 0:ntile, 0:16])
                rslice = ftabr[:].rearrange("p (t c) -> p t c", t=nt, c=12)[
                    :, g * 16:g * 16 + ntile, :]
                nc.scalar.copy(out=rslice, in_=pl3[:, 0:ntile, 16:28])
            # ---- supertiles
            for st in range(nst):
                Lp = Lps[st]
                G = Lp // 2
                t0 = st * B
                c0 = int(offs2[st])
                stg = sb.tile([96, G * P], bf16, tag="stg")
                nc.sync.dma_start(out=stg[:], in_=d_pk[:, c0:c0 + G * P])
                Ec = sb.tile([P, Lp * P], bf16, tag="Ec")
                E = sb.tile([P, Lp * P], bf16, tag="E")
                E4 = E[:].rearrange("p (l b c) -> p l b c", l=Lp, b=8, c=16)
                fsl = ftab[:].rearrange("p (t c) -> p t c", t=nt, c=16)[
                    :, t0:t0 + 8, :].rearrange("p b c -> p (b c)")
                for g4 in range(Lp // 4):
                    pt = ps.tile([P, 512], f32, tag="pt")
                    for i in range(2):
                        g = g4 * 2 + i
                        nc.tensor.matmul(out=pt[:, i * 256:(i + 1) * 256],
                                         lhsT=stg[:, g * P:(g + 1) * P],
                                         rhs=bd[:], start=True, stop=True)
                    nc.scalar.copy(out=Ec[:, g4 * 512:(g4 + 1) * 512], in_=pt[:])
                fb = fsl.unsqueeze(1).broadcast_to([P, Lp, P])
                nc.vector.tensor_tensor(
                    out=E[:].rearrange("p (l x) -> p l x", l=Lp, x=P),
                    in0=Ec[:].rearrange("p (l x) -> p l x", l=Lp, x=P),
                    in1=fb, op=AL.add)
                # scores: RP/RN per head (abs reduces over sign-grouped cols)
                RPN = []
                for h in range(2):
                    for pos in (True, False):
                        cw = npos[h] if pos else 5 - npos[h]
                        cb = 2 + 5 * h + (0 if pos else npos[h])
                        r = sb3.tile([P, Lp * 8], bf16, tag=f"r{h}{int(pos)}")
                        if cw > 0:
                            nc.vector.tensor_reduce(
                                out=r[:].rearrange("p (l b) -> p l b", l=Lp, b=8),
                                in_=E4[:, :, :, cb:cb + cw], axis=AX.X, op=AL.add,
                                apply_absolute_value=True)
                        else:
                            nc.vector.memset(r[:], 0.0)
                        RPN.append(r)
                for h in range(2):
                    q = sb3.tile([P, Lp * 8], bf16, tag=f"q{h}")
                    nc.vector.tensor_tensor(out=q[:], in0=RPN[2 * h][:],
                                            in1=RPN[2 * h + 1][:], op=AL.subtract)
                    s = sb3.tile([P, Lp * 8], bf16, tag=f"s{h}")
                    nc.vector.scalar_tensor_tensor(
                        out=s[:].rearrange("p (l b) -> p l b", l=Lp, b=8),
                        in0=E4[:, :, :, 12 + h], scalar=1.5, in1=q[:].rearrange(
                            "p (l b) -> p l b", l=Lp, b=8),
                        op0=AL.mult, op1=AL.add)
                    nc.scalar.activation(
                        out=E4[:, :, :, 12 + h],
                        in_=s[:].rearrange("p (l b) -> p l b", l=Lp, b=8),
                        func=AF.Exp, scale=0.4)
                # a1 scores
                vc = sb3.tile([P, Lp * 8], bf16, tag="vc")
                nc.scalar.copy(out=vc[:].rearrange("p (l b) -> p l b", l=Lp, b=8),
                               in_=E4[:, :, :, 14])
                pa = sb3.tile([P, Lp * 8 * 2], bf16, tag="pa")
                pa3 = pa[:].rearrange("p (l b c) -> p l b c", l=Lp, b=8, c=2)
                nc.scalar.activation(out=pa3, in_=E4[:, :, :, 0:2],
                                     func=AF.Prelu, alpha=NEG_GAT)
                for h in range(2):
                    nc.scalar.activation(out=E4[:, :, :, 14 + h],
                                         in_=pa3[:, :, :, h], func=AF.Exp,
                                         scale=float(sa[h]))
                # mask pads, weight by ex
                vb = vc[:].rearrange("p (l b) -> p l b", l=Lp, b=8
                                     ).unsqueeze(3).broadcast_to([P, Lp, 8, 4])
                nc.vector.tensor_tensor(out=E4[:, :, :, 12:16],
                                        in0=E4[:, :, :, 12:16], in1=vb, op=AL.mult)
                exd = E4[:, :, :, 12:14].unsqueeze(4).broadcast_to([P, Lp, 8, 2, 5])
                wd = E4[:, :, :, 2:12].rearrange("p l b (h f) -> p l b h f", h=2, f=5)
                nc.vector.tensor_tensor(out=wd, in0=wd, in1=exd, op=AL.mult)
                nc.vector.tensor_tensor(out=E4[:, :, :, 0:2], in0=E4[:, :, :, 0:2],
                                        in1=E4[:, :, :, 14:16], op=AL.mult)
                # tree reduce over slots
                e2v = E[:].rearrange("p (l q x) -> p l q x", l=Lp // 2, q=2, x=P)
                T1 = sb3.tile([P, (Lp // 2) * P], bf16, tag="T1")
                t1v = T1[:].rearrange("p (l x) -> p l x", l=Lp // 2, x=P)
                nc.vector.tensor_tensor(out=t1v, in0=e2v[:, :, 0, :],
                                        in1=e2v[:, :, 1, :], op=AL.add)
                t2s = T1[:].rearrange("p (l q x) -> p l q x", l=Lp // 4, q=2, x=P)
                T2 = sb3.tile([P, (Lp // 4) * P], bf16, tag="T2")
                t2v = T2[:].rearrange("p (l x) -> p l x", l=Lp // 4, x=P)
                nc.vector.tensor_tensor(out=t2v, in0=t2s[:, :, 0, :],
                                        in1=t2s[:, :, 1, :], op=AL.add)
                t3s = T2[:].rearrange("p (l q x) -> p l q x", l=Lp // 8, q=2, x=P)
                T3 = sb3.tile([P, (Lp // 8) * P], f32, tag="T3")
                t3v = T3[:].rearrange("p (l x) -> p l x", l=Lp // 8, x=P)
                nc.vector.tensor_tensor(out=t3v, in0=t3s[:, :, 0, :],
                                        in1=t3s[:, :, 1, :], op=AL.add)
                R = sb3.tile([P, P], f32, tag="R")
                nc.vector.tensor_reduce(
                    out=R[:].rearrange("p (b c) -> p b c", b=8, c=16),
                    in_=T3[:].rearrange("p (m b c) -> p b c m", m=Lp // 8, b=8, c=16),
                    axis=AX.X, op=AL.add)
                # normalize + residual + elu
                R3 = R[:].rearrange("p (b c) -> p b c", b=8, c=16)
                nc.vector.tensor_scalar_max(out=R3[:, :, 12:16],
                                            in0=R3[:, :, 12:16], scalar1=1e-30)
                REC = sb3.tile([P, 32], f32, tag="REC")
                rec3 = REC[:].rearrange("p (b c) -> p b c", b=8, c=4)
                nc.vector.reciprocal(out=rec3, in_=R3[:, :, 12:16])
                U = sb3.tile([P, 8 * 12], f32, tag="U")
                U3 = U[:].rearrange("p (b c) -> p b c", b=8, c=12)
                u_d = U3[:, :, 2:12].rearrange("p b (h f) -> p b h f", h=2, f=5)
                r_d = R3[:, :, 2:12].rearrange("p b (h f) -> p b h f", h=2, f=5)
                rec_d = rec3[:, :, 0:2].unsqueeze(3).broadcast_to([P, 8, 2, 5])
                nc.vector.tensor_tensor(out=u_d, in0=r_d, in1=rec_d, op=AL.mult)
                nc.vector.tensor_tensor(out=U3[:, :, 0:2], in0=R3[:, :, 0:2],
                                        in1=rec3[:, :, 2:4], op=AL.mult)
                fsl3 = ftab[:].rearrange("p (t c) -> p t c", t=nt, c=16)[
                    :, t0:t0 + 8, 0:12]
                nc.vector.tensor_tensor(out=U3, in0=U3, in1=fsl3, op=AL.subtract)
                ib = invat[:].unsqueeze(1).broadcast_to([P, 8, 12])
                nc.vector.tensor_tensor(out=U3, in0=U3, in1=ib, op=AL.mult)
                rsl3 = ftabr[:].rearrange("p (t c) -> p t c", t=nt, c=12)[
                    :, t0:t0 + 8, :]
                nc.vector.tensor_tensor(out=U3, in0=U3, in1=rsl3, op=AL.add)
                EM = sb3.tile([P, 8 * 12], f32, tag="EM")
                nc.vector.tensor_scalar_min(out=EM[:], in0=U[:], scalar1=0.0)
                nc.scalar.activation(out=EM[:], in_=EM[:], func=AF.Exp)
                nc.vector.tensor_scalar_max(out=U[:], in0=U[:], scalar1=0.0)
                nc.vector.scalar_tensor_tensor(
                    out=h1o[:, t0 * 12:(t0 + 8) * 12], in0=EM[:], scalar=-1.0,
                    in1=U[:], op0=AL.add, op1=AL.add)
            nc.sync.dma_start(out=d_h1o[:, :], in_=h1o[:])
    nc.compile()
    return nc


# =============================================================== launch2 (v1)
def build_launch2(nst, Ls, offs, nt):
    nchunk = nt // 8
    totc = int(offs[-1])
    nmc = nt * P // 512
    nc = bacc.Bacc("TRN2", target_bir_lowering=False, debug=False, num_devices=NCORE)
    d_pk = nc.dram_tensor("hde", [88, totc], bf16, kind="ExternalInput")
    d_lpk = nc.dram_tensor("h1l", [88, nchunk * P], bf16, kind="ExternalInput")
    d_np = nc.dram_tensor("npad", [P, nt], f32, kind="ExternalInput")
    d_at = nc.dram_tensor("attn4", [P, 4], bf16, kind="ExternalInput")
    d_b2e = nc.dram_tensor("bd2e", [88, 64], bf16, kind="ExternalInput")
    d_b2l = nc.dram_tensor("bd2l", [88, 64], bf16, kind="ExternalInput")
    d_hatt = nc.dram_tensor("hatt", [P, nt * 2], bf16, kind="ExternalInput")
    d_x = nc.dram_tensor("xpm", [P, nt * 8], bf16, kind="ExternalInput")
    d_w1 = nc.dram_tensor("w1", [14, 196], bf16, kind="ExternalInput")
    d_w2 = nc.dram_tensor("w2", [196, 196], bf16, kind="ExternalInput")
    d_w3 = nc.dram_tensor("w3", [196, 14], bf16, kind="ExternalInput")
    d_w4 = nc.dram_tensor("w4", [14, 1], bf16, kind="ExternalInput")
    d_b1 = nc.dram_tensor("b1", [196], f32, kind="ExternalInput")
    d_b2 = nc.dram_tensor("b2", [196], f32, kind="ExternalInput")
    d_b3 = nc.dram_tensor("b3", [14], f32, kind="ExternalInput")
    d_b4 = nc.dram_tensor("b4", [1], f32, kind="ExternalInput")
    d_out = nc.dram_tensor("out", [nmc, 512], f32, kind="ExternalOutput")
    with TileContext(nc) as tc:
        with tc.tile_pool(name="res", bufs=1) as res, \
             tc.tile_pool(name="sb", bufs=2) as sb, \
             tc.tile_pool(name="sb3", bufs=3) as sb3:
            ident = res.tile([P, P], bf16)
            make_identity(nc, ident[:])
            attn = res.tile([P, 4], bf16)
            nc.sync.dma_start(out=attn[:], in_=d_at[:, :])
            npad_t = res.tile([P, nt], f32)
            nc.sync.dma_start(out=npad_t[:], in_=d_np[:, :])
            b2e = res.tile([88, 64], bf16, tag="b2e")
            nc.sync.dma_start(out=b2e[:], in_=d_b2e[:, :])
            b2l = res.tile([88, 64], bf16, tag="b2l")
            nc.sync.dma_start(out=b2l[:], in_=d_b2l[:, :])
            hatt = res.tile([P, nt * 2], bf16)
            nc.sync.dma_start(out=hatt[:], in_=d_hatt[:, :])
            xpm = res.tile([P, nt * 8], bf16)
            nc.sync.dma_start(out=xpm[:], in_=d_x[:, :])
            ftab2 = res.tile([P, nt * 4], bf16)
            ftab2r = res.tile([P, nt * 4], f32)
            h2o = res.tile([P, nt * 4], f32)
            w1 = res.tile([14, 196], bf16, tag="w1")
            nc.sync.dma_start(out=w1[:], in_=d_w1[:, :])
            w2a = res.tile([P, 196], bf16, tag="w2a")
            nc.sync.dma_start(out=w2a[:], in_=d_w2[0:128, :])
            w2b = res.tile([68, 196], bf16, tag="w2b")
            nc.sync.dma_start(out=w2b[:], in_=d_w2[128:196, :])
            w3a = res.tile([P, 14], bf16, tag="w3a")
            nc.sync.dma_start(out=w3a[:], in_=d_w3[0:128, :])
            w3b = res.tile([68, 14], bf16, tag="w3b")
            nc.sync.dma_start(out=w3b[:], in_=d_w3[128:196, :])
            w4 = res.tile([14, 1], bf16, tag="w4")
            nc.sync.dma_start(out=w4[:], in_=d_w4[:, :])
            b1ca = res.tile([P, 1], f32, tag="b1ca")
            nc.sync.dma_start(out=b1ca[:], in_=d_b1[0:128, None])
            b1cb = res.tile([68, 1], f32, tag="b1cb")
            nc.sync.dma_start(out=b1cb[:], in_=d_b1[128:196, None])
            b2ca = res.tile([P, 1], f32, tag="b2ca")
            nc.sync.dma_start(out=b2ca[:], in_=d_b2[0:128, None])
            b2cb = res.tile([68, 1], f32, tag="b2cb")
            nc.sync.dma_start(out=b2cb[:], in_=d_b2[128:196, None])
            b3c = res.tile([14, 1], f32, tag="b3c")
            nc.sync.dma_start(out=b3c[:], in_=d_b3[:, None])
            b4c = res.tile([1, 1], f32, tag="b4c")
            nc.sync.dma_start(out=b4c[:], in_=d_b4[:, None])
            with tc.tile_pool(name="ps", bufs=2, space="PSUM") as ps, \
                 tc.tile_pool(name="psl", bufs=1, space="PSUM") as psl:
                for ch in range(nchunk):
                    stg = sb.tile([88, P], bf16, tag="lstg")
                    nc.sync.dma_start(out=stg[:], in_=d_lpk[:, ch * P:(ch + 1) * P])
                    pmm = psl.tile([64, P], f32, tag="lmm")
                    nc.tensor.matmul(out=pmm[:], lhsT=b2l[:], rhs=stg[:],
                                     start=True, stop=True)
                    cp = sb.tile([64, P], bf16, tag="lcp")
                    nc.vector.tensor_copy(out=cp[:], in_=pmm[:])
                    pt = psl.tile([P, 64], bf16, tag="ltt")
                    nc.tensor.transpose(out=pt[:], in_=cp[:], identity=ident[0:64, 0:64])
                    psl_ap = pt[:].rearrange("p (a c) -> p a c", a=8, c=8)
                    f2s = ftab2[:].rearrange("p (t c) -> p t c", t=nt, c=4)[:, ch * 8:(ch + 1) * 8, :]
                    nc.scalar.copy(out=f2s, in_=psl_ap[:, :, 0:4])
                    f2r = ftab2r[:].rearrange("p (t c) -> p t c", t=nt, c=4)[:, ch * 8:(ch + 1) * 8, :]
                    nc.scalar.copy(out=f2r, in_=psl_ap[:, :, 4:8])
                for st in range(nst):
                    L = Ls[st]
                    ncols = B * L * 16
                    stg = sb.tile([88, ncols], bf16, tag="estg")
                    nc.sync.dma_start(out=stg[:], in_=d_pk[:, int(offs[st]):int(offs[st]) + ncols])
                    G = sb.tile([P, ncols // 2], bf16, tag="G")
                    emit_project(nc, sb, ps, stg[:], b2e[:], ident[:], G, ncols, gcopy="vec", sp=8)
                    t0 = st * B
                    fd_ap = ftab2[:].rearrange("p (t c) -> p t c", t=nt, c=4)[:, t0:t0 + B, :]
                    emit_gat_supertile(nc, sb, sb3, G, L, 4, 2, fd_ap,
                                       ftab2r[:, t0 * 4:(t0 + B) * 4],
                                       npad_t[:, t0:t0 + B], attn[:],
                                       0, 2, False, h2o[:, t0 * 4:(t0 + B) * 4], gsp=8)
                for mc in range(nmc):
                    t0 = mc * 4
                    m14 = sb.tile([P, 4 * 14], bf16, tag="m14")
                    m143 = m14[:].rearrange("p (t c) -> p t c", t=4, c=14)
                    ha = hatt[:].rearrange("p (t c) -> p t c", t=nt, c=2)[:, t0:t0 + 4, :]
                    nc.vector.tensor_copy(out=m143[:, :, 0:2], in_=ha)
                    h2s = h2o[:].rearrange("p (t c) -> p t c", t=nt, c=4)[:, t0:t0 + 4, :]
                    nc.vector.tensor_copy(out=m143[:, :, 2:6], in_=h2s)
                    xs = xpm[:].rearrange("p (t c) -> p t c", t=nt, c=8)[:, t0:t0 + 4, :]
                    nc.vector.tensor_copy(out=m143[:, :, 6:14], in_=xs)
                    r0 = sb.tile([14, 512], bf16, tag="r0")
                    for b in range(4):
                        ptm = psl.tile([14, P], bf16, tag="ltt")
                        nc.tensor.transpose(out=ptm[:], in_=m14[:, b * 14:(b + 1) * 14],
                                            identity=ident[:])
                        nc.vector.tensor_copy(out=r0[:, b * P:(b + 1) * P], in_=ptm[:])
                    p1a = psl.tile([P, 512], f32, tag="pA")
                    nc.tensor.matmul(out=p1a[:], lhsT=w1[:, 0:128], rhs=r0[:], start=True, stop=True)
                    p1b = psl.tile([68, 512], f32, tag="pB")
                    nc.tensor.matmul(out=p1b[:], lhsT=w1[:, 128:196], rhs=r0[:], start=True, stop=True)
                    r1a = sb.tile([P, 512], bf16, tag="r1a")
                    nc.scalar.activation(out=r1a[:], in_=p1a[:], func=AF.Prelu,
                                         alpha=NEG_MLP, bias=b1ca[:])
                    r1b = sb.tile([68, 512], bf16, tag="r1b")
                    nc.scalar.activation(out=r1b[:], in_=p1b[:], func=AF.Prelu,
                                         alpha=NEG_MLP, bias=b1cb[:])
                    p2a = psl.tile([P, 512], f32, tag="pA")
                    nc.tensor.matmul(out=p2a[:], lhsT=w2a[:, 0:128], rhs=r1a[:], start=True, stop=False)
                    nc.tensor.matmul(out=p2a[:], lhsT=w2b[:, 0:128], rhs=r1b[:], start=False, stop=True)
                    p2b = psl.tile([68, 512], f32, tag="pB")
                    nc.tensor.matmul(out=p2b[:], lhsT=w2a[:, 128:196], rhs=r1a[:], start=True, stop=False)
                    nc.tensor.matmul(out=p2b[:], lhsT=w2b[:, 128:196], rhs=r1b[:], start=False, stop=True)
                    r2a = sb.tile([P, 512], bf16, tag="r2a")
                    nc.scalar.activation(out=r2a[:], in_=p2a[:], func=AF.Prelu,
                                         alpha=NEG_MLP, bias=b2ca[:])
                    r2b = sb.tile([68, 512], bf16, tag="r2b")
                    nc.scalar.activation(out=r2b[:], in_=p2b[:], func=AF.Prelu,
                                         alpha=NEG_MLP, bias=b2cb[:])
                    p3 = psl.tile([14, 512], f32, tag="pA")
                    nc.tensor.matmul(out=p3[:], lhsT=w3a[:], rhs=r2a[:], start=True, stop=False)
                    nc.tensor.matmul(out=p3[:], lhsT=w3b[:], rhs=r2b[:], start=False, stop=True)
                    r3 = sb.tile([14, 512], bf16, tag="r3")
                    nc.scalar.activation(out=r3[:], in_=p3[:], func=AF.Prelu,
                                         alpha=NEG_MLP, bias=b3c[:])
                    po = psl.tile([1, 512], f32, tag="pB")
                    nc.tensor.matmul(out=po[:], lhsT=w4[:], rhs=r3[:], start=True, stop=True)
                    sg = sb.tile([1, 512], f32, tag="sg")
                    nc.scalar.activation(out=sg[:], in_=po[:], func=AF.Sigmoid, bias=b4c[:])
                    nc.sync.dma_start(out=d_out[mc:mc + 1, :], in_=sg[:])
    nc.compile()
    return nc


# ---- v1 helpers still used by launch2
def emit_project(nc, sb, ps, stg, bd_t, ident, G, ncols, gcopy="act", sp=16):
    M = 8 * sp
    for ci, c0 in enumerate(range(0, ncols, 512)):
        cw = min(512, ncols - c0)
        pmm = ps.tile([M, 512], f32, tag="mmout")
        nc.tensor.matmul(out=pmm[:, :cw], lhsT=bd_t, rhs=stg[:, c0:c0 + cw],
                         start=True, stop=True)
        cp = sb.tile([M, 512], bf16, tag="mmcopy")
        if ci % 2 == 0:
            nc.vector.tensor_copy(out=cp[:, :cw], in_=pmm[:, :cw])
        else:
            nc.scalar.copy(out=cp[:, :cw], in_=pmm[:, :cw])
        for k0 in range(0, cw, P):
            pt = ps.tile([P, M], bf16, tag="tout")
            nc.tensor.transpose(out=pt[:], in_=cp[:, k0:k0 + P], identity=ident[0:M, 0:M])
            q0 = (c0 + k0) // P * 8
            if gcopy == "act":
                nc.scalar.copy(out=G[:, q0 * sp:(q0 + 8) * sp], in_=pt[:])
            else:
                nc.vector.tensor_copy(out=G[:, q0 * sp:(q0 + 8) * sp], in_=pt[:])


def emit_gat_supertile(nc, sb, sb3, G, L, C, NH, fd_ap, ftr_ap, npad_ap, attn_ap,
                       d_off, d_F, a1, out_ap, gsp=16):
    BL = B * L
    nd = 2 if a1 else 0
    nrh = NH - nd
    G4 = G[:].rearrange("p (b l c) -> p b l c", b=B, l=L, c=gsp)[:, :, :, 0:C]
    fdb = fd_ap.unsqueeze(2).broadcast_to([P, B, L, C])
    e = sb.tile([P, BL * C], bf16, tag="bA")
    e4 = e[:].rearrange("p (b l c) -> p b l c", b=B, l=L, c=C)
    z = sb.tile([P, BL * C], bf16, tag="bB")
    z4 = z[:].rearrange("p (b l c) -> p b l c", b=B, l=L, c=C)
    zA = sb.tile([P, BL * C], bf16, tag="bC")
    zA4 = zA[:].rearrange("p (b l c) -> p b l c", b=B, l=L, c=C)
    atb = attn_ap.unsqueeze(1).unsqueeze(1).broadcast_to([P, B, L, C])
    ex = sb3.tile([P, BL * NH], bf16, tag="ex")
    ex4 = ex[:].rearrange("p (b l h) -> p b l h", b=B, l=L, h=NH)
    sd = sb3.tile([P, BL * nrh], f32, tag="sd")
    sd4 = sd[:].rearrange("p (b l h) -> p b l h", b=B, l=L, h=nrh)
    hb = B // 2
    for hf in range(2):
        b0, b1 = hf * hb, (hf + 1) * hb
        nc.vector.tensor_tensor(out=e4[:, b0:b1], in0=G4[:, b0:b1],
                                in1=fdb[:, b0:b1], op=AL.add)
        nc.scalar.activation(out=z4[:, b0:b1], in_=e4[:, b0:b1],
                             func=AF.Prelu, alpha=NEG_GAT)
        nc.vector.tensor_tensor(out=zA4[:, b0:b1], in0=z4[:, b0:b1],
                                in1=atb[:, b0:b1], op=AL.mult)
        for h in range(nrh):
            nc.vector.tensor_reduce(out=sd4[:, b0:b1, :, h],
                                    in_=zA4[:, b0:b1, :, d_off + h::2],
                                    axis=AX.X, op=AL.add)
        if a1:
            nc.scalar.activation(out=ex4[:, b0:b1, :, 0:2],
                                 in_=zA4[:, b0:b1, :, 0:2], func=AF.Exp)
        nc.scalar.activation(out=ex4[:, b0:b1, :, nd:NH],
                             in_=sd4[:, b0:b1], func=AF.Exp)
    den = sb3.tile([P, B * NH], f32, tag="den")
    den3 = den[:].rearrange("p (b h) -> p b h", b=B, h=NH)
    exT = ex4.rearrange("p b l h -> p b h l")
    nc.vector.tensor_reduce(out=den3, in_=exT, axis=AX.X, op=AL.add)
    w = sb.tile([P, BL * C], bf16, tag="bB")
    w4 = w[:].rearrange("p (b l c) -> p b l c", b=B, l=L, c=C)
    if a1:
        nc.vector.tensor_tensor(out=w4[:, :, :, 0:2], in0=G4[:, :, :, 0:2],
                                in1=ex4[:, :, :, 0:2], op=AL.mult)
    exd = ex4[:, :, :, nd:NH].unsqueeze(3).broadcast_to([P, B, L, d_F, nrh])
    G5 = G4[:, :, :, d_off:C].rearrange("p b l (f h) -> p b l f h", f=d_F, h=nrh)
    w5 = w4[:, :, :, d_off:C].rearrange("p b l (f h) -> p b l f h", f=d_F, h=nrh)
    nc.vector.tensor_tensor(out=w5, in0=G5, in1=exd, op=AL.mult)
    zp = sb3.tile([P, B * C], bf16, tag="zp")
    zp3 = zp[:].rearrange("p (b c) -> p b c", b=B, c=C)
    nc.scalar.activation(out=zp3, in_=fd_ap, func=AF.Prelu, alpha=NEG_GAT)
    zpA = sb3.tile([P, B * C], bf16, tag="zpA")
    zpA3 = zpA[:].rearrange("p (b c) -> p b c", b=B, c=C)
    atb2 = attn_ap.unsqueeze(1).broadcast_to([P, B, C])
    nc.vector.tensor_tensor(out=zpA3, in0=zp3, in1=atb2, op=AL.mult)
    exp_p = sb3.tile([P, B * NH], bf16, tag="exp_p")
    exp_p3 = exp_p[:].rearrange("p (b h) -> p b h", b=B, h=NH)
    spd = sb3.tile([P, B * nrh], f32, tag="spd")
    spd3 = spd[:].rearrange("p (b h) -> p b h", b=B, h=nrh)
    for h in range(nrh):
        nc.vector.tensor_reduce(out=spd3[:, :, h], in_=zpA3[:, :, d_off + h::2],
                                axis=AX.X, op=AL.add)
    if a1:
        nc.scalar.activation(out=exp_p3[:, :, 0:2], in_=zpA3[:, :, 0:2], func=AF.Exp)
    nc.scalar.activation(out=exp_p3[:, :, nd:NH], in_=spd3, func=AF.Exp)
    padm = sb3.tile([P, B * NH], f32, tag="padm")
    padm3 = padm[:].rearrange("p (b h) -> p b h", b=B, h=NH)
    npb = npad_ap.unsqueeze(2).broadcast_to([P, B, NH])
    nc.vector.tensor_tensor(out=padm3, in0=exp_p3, in1=npb, op=AL.mult)
    nc.vector.tensor_tensor(out=den[:], in0=den[:], in1=padm[:], op=AL.subtract)
    nc.vector.tensor_scalar_max(out=den[:], in0=den[:], scalar1=1e-30)
    rec = sb3.tile([P, B * NH], f32, tag="rec")
    nc.vector.reciprocal(out=rec[:], in_=den[:])
    rec3 = rec[:].rearrange("p (b h) -> p b h", b=B, h=NH)
    rstn = sb3.tile([P, B * C], f32, tag="rstn")
    rstn3 = rstn[:].rearrange("p (b c) -> p b c", b=B, c=C)
    wT = w4.rearrange("p b l c -> p b c l")
    nc.vector.tensor_reduce(out=rstn3, in_=wT, axis=AX.X, op=AL.add)
    if a1:
        nc.vector.tensor_tensor(out=rstn3[:, :, 0:2], in0=rstn3[:, :, 0:2],
                                in1=rec3[:, :, 0:2], op=AL.mult)
        rr4 = rec3[:, :, nd:NH].unsqueeze(2).broadcast_to([P, B, d_F, nrh])
        rd4 = rstn3[:, :, d_off:C].rearrange("p b (f h) -> p b f h", f=d_F, h=nrh)
        nc.vector.tensor_tensor(out=rd4, in0=rd4, in1=rr4, op=AL.mult)
    else:
        rr4 = rec3.unsqueeze(2).broadcast_to([P, B, d_F, nrh])
        rd4 = rstn3.rearrange("p b (f h) -> p b f h", f=d_F, h=nrh)
        nc.vector.tensor_tensor(out=rd4, in0=rd4, in1=rr4, op=AL.mult)
    nc.vector.tensor_tensor(out=rstn[:], in0=rstn[:], in1=ftr_ap, op=AL.add)
    tmin = sb3.tile([P, B * C], f32, tag="tmin")
    nc.vector.tensor_scalar_min(out=tmin[:], in0=rstn[:], scalar1=0.0)
    epx = sb3.tile([P, B * C], f32, tag="epx")
    nc.scalar.activation(out=epx[:], in_=tmin[:], func=AF.Exp)
    nc.vector.tensor_scalar_max(out=rstn[:], in0=rstn[:], scalar1=0.0)
    nc.vector.tensor_tensor(out=rstn[:], in0=rstn[:], in1=epx[:], op=AL.add)
    nc.vector.tensor_scalar_add(out=out_ap, in0=rstn[:], scalar1=-1.0)


# ================================================================== weights
def d2_rowperm(w):
    out = np.zeros_like(w)
    for f in range(5):
        for h in range(2):
            out[2 * f + h] = w[5 * h + f]
    return out


def d2_colperm4(v):
    out = np.zeros_like(v)
    for f in range(2):
        for h in range(2):
            out[..., 2 * f + h] = v[..., 2 * h + f]
    return out


_cache = {}


def kernel(**inputs):
    x = np.asarray(inputs['x'], np.float32)
    src = np.asarray(inputs['src'], np.int32)
    dst = np.asarray(inputs['dst'], np.int32)
    n = x.shape[0]

    scheds, nst, Ls, offs, Lps, offs2 = build_schedule(dst, n)
    nt = scheds[0]['nt']
    nloc = scheds[0]['nloc']
    totc = int(offs[-1])
    totc2 = int(offs2[-1])
    ngl = -(-nt // 16)

    bd2, ml16, inva, npos, sa, perms = build_l1_weights(inputs)
    invat = np.tile(inva.reshape(1, 12), (P, 1)).astype(np.float32)

    # ---- launch2 weights (v1)
    ws2 = d2_rowperm(d2_colperm4(np.asarray(inputs['d2_Wsrc'], np.float32)))
    bs2 = d2_colperm4(np.asarray(inputs['d2_bsrc'], np.float32))
    wd2 = d2_rowperm(d2_colperm4(np.asarray(inputs['d2_Wdst'], np.float32)))
    bdst2 = d2_colperm4(np.asarray(inputs['d2_bdst'], np.float32))
    wr2 = d2_rowperm(d2_colperm4(np.asarray(inputs['d2_Wres'], np.float32)))
    bias2 = d2_colperm4(np.asarray(inputs['d2_bias'], np.float32))
    bd2e = blockdiag(ws2, bs2, 11, sp=8)
    bd2l = blockdiag(np.concatenate([wd2, wr2], axis=1),
                     np.concatenate([bdst2, bias2]), 11, sp=8)
    attn4 = np.zeros(4, np.float32)
    for f in range(2):
        for h in range(2):
            attn4[2 * f + h] = np.asarray(inputs['d2_attn'])[h, f]
    attn4_t = np.tile(attn4.astype(BF), (P, 1))
    w1p = np.asarray(inputs['W1'], np.float32).copy()
    for f in range(2):
        for h in range(2):
            w1p[2 + 2 * f + h] = np.asarray(inputs['W1'])[2 + 2 * h + f]

    key = (n, len(src), nst, tuple(Ls), tuple(Lps), npos, sa)
    if key not in _cache:
        _cache[key] = (build_launch1_v2(nst, Lps, offs2, nt, ngl, npos, sa),
                       build_launch2(nst, Ls, offs, nt))
    nc1, nc2 = _cache[key]

    in1, core_meta = [], []
    for s in scheds:
        eo, pos_sorted, rank = edge_pos(s)
        a_e, col_e, npad = edge_slot_cols_old(s, Ls, offs, eo, pos_sorted, rank)
        rb2, col2 = edge_cols_v2(s, offs2, eo, pos_sorted, rank)
        core_meta.append((s, eo, a_e, col_e, npad))
        x5e = pack_edges_v2(x[src[s['em']], :5], eo, rb2, col2, totc2)
        orig = s['order']
        valid = orig < nloc
        xl = np.zeros((nt * P, 5), np.float32)
        xl[valid] = x[s['core'] * nloc + orig[valid], :5]
        in1.append(dict(x5e=x5e, x5l=pack_xl16(xl, valid, nt, ngl),
                        bd2=bd2, ml16=ml16, inva=invat))
    r1 = run_bass_kernel_spmd(nc1, in1, core_ids=list(range(NCORE)))
    t1 = r1.exec_time_ns or 0

    # device h1 col order: [a1h0, a1h1, (h0: perm0), (h1: perm1)]
    # -> old convention: hatt 0:2 ; hdef col 2 + 2f + h = std[5h+f]
    hdef_g = np.zeros((n, 10), np.float32)
    hatt_all, h1_old_all = [], []
    for ci, s in enumerate(scheds):
        h1 = r1.results[ci]['h1o'].reshape(P, nt, 12).transpose(1, 0, 2).reshape(nt * P, 12)
        old = np.zeros((nt * P, 10), np.float32)
        for h in range(2):
            for j, f in enumerate(perms[h]):
                old[:, 2 * f + h] = h1[:, 2 + 5 * h + j]
        h1_old_all.append(old)
        orig = s['order']
        valid = orig < nloc
        hdef_g[s['core'] * nloc + orig[valid]] = old[valid]
        hatt_all.append(h1[:, 0:2])

    in2 = []
    for ci, (s, eo, a_e, col_e, npad) in enumerate(core_meta):
        hde = pack_edges_old(hdef_g[src[s['em']]], eo, a_e, col_e, totc, 11)
        orig = s['order']
        valid = orig < nloc
        xl8 = np.zeros((nt * P, 8), np.float32)
        xl8[valid] = x[s['core'] * nloc + orig[valid], :]
        in2.append(dict(hde=hde, h1l=pack_local(h1_old_all[ci], 11, nt),
                        npad=npad, attn4=attn4_t, bd2e=bd2e, bd2l=bd2l,
                        hatt=pm(hatt_all[ci], nt).astype(BF),
                        xpm=pm(xl8, nt).astype(BF),
                        w1=w1p.astype(BF),
                        w2=np.asarray(inputs['W2'], np.float32).astype(BF),
                        w3=np.asarray(inputs['W3'], np.float32).astype(BF),
                        w4=np.asarray(inputs['W4'], np.float32).astype(BF),
                        b1=np.asarray(inputs['b1'], np.float32),
                        b2=np.asarray(inputs['b2'], np.float32),
                        b3=np.asarray(inputs['b3'], np.float32),
                        b4=np.asarray(inputs['b4'], np.float32)))
    r2 = run_bass_kernel_spmd(nc2, in2, core_ids=list(range(NCORE)))
    t2 = r2.exec_time_ns or 0

    out = np.zeros((n, 1), np.float32)
    for ci, s in enumerate(scheds):
        y = r2.results[ci]['out'].reshape(nt * P)
        orig = s['order']
        valid = orig < nloc
        out[s['core'] * nloc + orig[valid], 0] = y[valid]
    kernel.last_exec_ns = t1 + t2
    kernel.last_t12 = (t1, t2)
    kernel.last_results = (r1, r2)
    return out


# revision 9
# speedup vs baseline: 1.3975x; 1.3975x over previous
"""GATv2 x3 + MLP (nn_GAT) on trn2, 8 NeuronCores.

v2 launch1: stationary-swapped projection (edge-stream is the matmul
stationary operand, block-diag weights are the moving operand) writes
node-major bf16 tiles straight into PSUM -- no PE transposes, no
PSUM->SBUF copy pass.  Softmax scores use the lrelu decomposition
lrelu(v) = 0.6 v + 0.4|v|: the linear term rides along the projection as
two extra columns, the |.| term is two abs-reduces over sign-grouped
columns.  Per-edge exp() columns share the E tile so one tree reduction
over slots produces both the weighted sums and the softmax denominators.
Launch2 (d2 GAT + MLP) still the v1 transpose pipeline.
"""
import sys
sys.path.insert(0, '/opt/trn_rl_repo')
import numpy as np
import ml_dtypes

import concourse.bass as bass
import concourse.mybir as mybir
from concourse import bacc
from concourse.tile import TileContext
from concourse.bass_utils import run_bass_kernel_spmd
from concourse.masks import make_identity

bf16 = mybir.dt.bfloat16
f32 = mybir.dt.float32
BF = ml_dtypes.bfloat16
AL = mybir.AluOpType
AF = mybir.ActivationFunctionType
AX = mybir.AxisListType

NCORE = 8
P = 128
B = 8
NEG_GAT = 0.2
NEG_MLP = 0.01


# ================================================================= host prep
def build_schedule(dst, n):
    nloc = n // NCORE
    core_of = dst // nloc
    scheds = []
    for c in range(NCORE):
        em = np.where(core_of == c)[0]
        ldst = dst[em] - c * nloc
        deg = np.bincount(ldst, minlength=nloc)
        nt = -(-nloc // P)
        nt = -(-nt // B) * B
        degp = np.concatenate([deg, np.zeros(nt * P - nloc, np.int64)])
        order = np.argsort(-degp, kind='stable')
        pos_of = np.empty_like(order)
        pos_of[order] = np.arange(len(order))
        scheds.append(dict(core=c, em=em, ldst=ldst, deg=degp, order=order,
                           pos_of=pos_of, nt=nt, nloc=nloc))
    nt = scheds[0]['nt']
    nst = nt // B
    Ls = []
    for st in range(nst):
        L = 1
        for s in scheds:
            L = max(L, int(s['deg'][s['order'][st * B * P]]))
        Ls.append(L)
    # old (launch2) stream offsets
    offs = np.concatenate([[0], np.cumsum([B * L * 16 for L in Ls])]).astype(np.int64)
    # v2 (launch1) stream: Lp multiple of 8, [96, (Lp/2)*128] blocks
    Lps = [-(-L // 8) * 8 for L in Ls]
    offs2 = np.concatenate([[0], np.cumsum([(Lp // 2) * P for Lp in Lps])]).astype(np.int64)
    return scheds, nst, Ls, offs, Lps, offs2


def edge_pos(s):
    """Common per-edge placement: sorted-position, rank within node."""
    pos_e = s['pos_of'][s['ldst']]
    eo = np.lexsort((np.arange(len(pos_e)), pos_e))
    pos_sorted = pos_e[eo]
    starts = np.concatenate([[0], np.cumsum(s['deg'][s['order']])])
    rank = np.arange(len(eo)) - starts[pos_sorted]
    return eo, pos_sorted, rank


def edge_slot_cols_old(s, Ls, offs, eo, pos_sorted, rank):
    """v1 layout for launch2: q = b*L + l, 16-col groups, 8-row interleave."""
    st_of = (pos_sorted // P) // B
    L_e = np.asarray(Ls)[st_of]
    q_e = ((pos_sorted // P) % B) * L_e + rank
    col_e = (offs[st_of] + (q_e // 8) * P + (pos_sorted % P)).astype(np.int64)
    a_e = (q_e % 8).astype(np.int64)
    npad = np.zeros((P, s['nt']), np.float32)
    for t in range(s['nt']):
        L = Ls[t // B]
        npad[:, t] = L - s['deg'][s['order'][t * P:(t + 1) * P]]
    return a_e, col_e, npad


def edge_cols_v2(s, offs2, eo, pos_sorted, rank):
    """v2 layout for launch1: col = offs2[st] + (l//2)*128 + p,
    row-base = (l%2)*48 + b*6."""
    st_of = (pos_sorted // P) // B
    b_e = (pos_sorted // P) % B
    p_e = pos_sorted % P
    col_e = (offs2[st_of] + (rank // 2) * P + p_e).astype(np.int64)
    rb_e = ((rank % 2) * 48 + b_e * 6).astype(np.int64)
    return rb_e, col_e


def pack_edges_old(feats, eo, a_e, col_e, totc, nrow):
    pk = np.zeros((8 * nrow, totc), BF)
    fe = feats[eo].astype(BF)
    for f in range(nrow - 1):
        pk[a_e * nrow + f, col_e] = fe[:, f]
    pk[a_e * nrow + (nrow - 1), col_e] = BF(1.0)
    return pk


def pack_edges_v2(feats, eo, rb_e, col_e, totc2):
    pk = np.zeros((96, totc2), BF)
    fe = feats[eo].astype(BF)
    for f in range(5):
        pk[rb_e + f, col_e] = fe[:, f]
    pk[rb_e + 5, col_e] = BF(1.0)
    return pk


def pack_local(vals, nrow, nt):
    pk = np.zeros((8 * nrow, (nt // 8) * P), BF)
    nodes = np.arange(nt * P)
    a = (nodes // P) % 8
    col = (nodes // (8 * P)) * P + nodes % P
    v = vals.astype(BF)
    for f in range(nrow - 1):
        pk[a * nrow + f, col] = v[:, f]
    pk[a * nrow + (nrow - 1), col] = BF(1.0)
    return pk


def pack_xl16(vals, valid, nt, ngl):
    """[96, ngl*128]: tile t -> block t%16 rows (t%16)*6+f, cols (t//16)*128+p."""
    pk = np.zeros((96, ngl * P), BF)
    nodes = np.arange(nt * P)
    t = nodes // P
    p = nodes % P
    row_b = (t % 16) * 6
    col = (t // 16) * P + p
    v = vals.astype(BF)
    for f in range(5):
        pk[row_b + f, col] = v[:, f]
    pk[row_b + 5, col] = np.where(valid, BF(1.0), BF(0.0))
    return pk


def blockdiag(w, bias, nrow, sp=16):
    bd = np.zeros((8 * nrow, 8 * sp), np.float32)
    k = w.shape[1]
    for a in range(8):
        bd[a * nrow:a * nrow + w.shape[0], a * sp:a * sp + k] = w
        bd[a * nrow + nrow - 1, a * sp:a * sp + k] = bias
    return bd.astype(BF)


def pm(vals, nt):
    d = vals.shape[1]
    return np.ascontiguousarray(
        vals.reshape(nt, P, d).transpose(1, 0, 2).reshape(P, nt * d))


# ======================================================== v2 weight builders
def build_l1_weights(inp):
    """Column map c: 0,1 a1(|a|-folded); 2+5h+j d1 (h, perm_h[j]) |a|-folded;
    12,13 sigma-tilde(h); 14 VALID; 15 zero. Returns M6src, Mdst, Mres, metadata."""
    aW = np.asarray(inp['a1_Wsrc'], np.float64)
    ab = np.asarray(inp['a1_bsrc'], np.float64)
    aWd = np.asarray(inp['a1_Wdst'], np.float64)
    abd = np.asarray(inp['a1_bdst'], np.float64)
    aat = np.asarray(inp['a1_attn'], np.float64)[:, 0]      # [2]
    aWr = np.asarray(inp['a1_Wres'], np.float64)
    abr = np.asarray(inp['a1_bias'], np.float64)
    dW = np.asarray(inp['d1_Wsrc'], np.float64)
    db = np.asarray(inp['d1_bsrc'], np.float64)
    dWd = np.asarray(inp['d1_Wdst'], np.float64)
    dbd = np.asarray(inp['d1_bdst'], np.float64)
    dat = np.asarray(inp['d1_attn'], np.float64)            # [2,5]
    dWr = np.asarray(inp['d1_Wres'], np.float64)
    dbr = np.asarray(inp['d1_bias'], np.float64)

    sa = np.sign(aat)
    sa[sa == 0] = 1.0
    aa = np.abs(aat)
    aa[aa == 0] = 1e-12
    perms, nposs = [], []
    for h in range(2):
        fpos = [f for f in range(5) if dat[h, f] > 0]
        fneg = [f for f in range(5) if dat[h, f] <= 0]
        perms.append(fpos + fneg)
        nposs.append(len(fpos))
    ad = np.abs(dat)
    ad[ad == 0] = 1e-12

    def mk(W, bvec, Wa, ba, sig):
        """[6,16] src- or dst-side column matrix (sig: include sigma cols)."""
        M = np.zeros((6, 16), np.float64)
        for h in range(2):
            M[0:5, h] = aa[h] * Wa[:, h]
            M[5, h] = aa[h] * ba[h]
            for j, f in enumerate(perms[h]):
                M[0:5, 2 + 5 * h + j] = ad[h, f] * W[:, 5 * h + f]
                M[5, 2 + 5 * h + j] = ad[h, f] * bvec[5 * h + f]
            if sig:
                for f in range(5):
                    M[0:5, 12 + h] += dat[h, f] * W[:, 5 * h + f]
                    M[5, 12 + h] += dat[h, f] * bvec[5 * h + f]
        return M

    M6src = mk(dW, db, aW, ab, True)
    M6src[5, 14] = 1.0          # VALID indicator from the ones-row
    Mdst = mk(dWd, dbd, aWd, abd, True)
    Mres = np.zeros((6, 12), np.float64)
    for h in range(2):
        Mres[0:5, h] = aWr[:, h]
        Mres[5, h] = abr[h]
        for j, f in enumerate(perms[h]):
            Mres[0:5, 2 + 5 * h + j] = dWr[:, 5 * h + f]
            Mres[5, 2 + 5 * h + j] = dbr[5 * h + f]

    inva = np.zeros(12, np.float32)
    inva[0] = 1.0 / aa[0]
    inva[1] = 1.0 / aa[1]
    for h in range(2):
        for j, f in enumerate(perms[h]):
            inva[2 + 5 * h + j] = 1.0 / ad[h, f]

    # moving operand [96, 256]: row (lp*48 + b*6 + f), col (lp*128 + b*16 + c)
    bd2 = np.zeros((96, 256), np.float32)
    for lp in range(2):
        for b in range(8):
            bd2[lp * 48 + b * 6: lp * 48 + b * 6 + 6,
                lp * 128 + b * 16: lp * 128 + b * 16 + 16] = M6src
    # local projection moving operand [96, 448]: 16 blocks of [6, 28]
    Mloc = np.concatenate([Mdst, Mres], axis=1)  # [6, 28]
    ml16 = np.zeros((96, 448), np.float32)
    for tt in range(16):
        ml16[tt * 6: tt * 6 + 6, tt * 28: tt * 28 + 28] = Mloc
    return (bd2.astype(BF), ml16.astype(BF), inva,
            tuple(nposs), (float(sa[0]), float(sa[1])), perms)


# ============================================================ launch1 v2
def build_launch1_v2(nst, Lps, offs2, nt, ngl, npos, sa):
    totc2 = int(offs2[-1])
    nc = bacc.Bacc("TRN2", target_bir_lowering=False, debug=False, num_devices=NCORE)
    d_pk = nc.dram_tensor("x5e", [96, totc2], bf16, kind="ExternalInput")
    d_xl = nc.dram_tensor("x5l", [96, ngl * P], bf16, kind="ExternalInput")
    d_bd = nc.dram_tensor("bd2", [96, 256], bf16, kind="ExternalInput")
    d_ml = nc.dram_tensor("ml16", [96, 448], bf16, kind="ExternalInput")
    d_inva = nc.dram_tensor("inva", [P, 12], f32, kind="ExternalInput")
    d_h1o = nc.dram_tensor("h1o", [P, nt * 12], f32, kind="ExternalOutput")
    with TileContext(nc) as tc, nc.allow_low_precision("bf16 GAT partials"):
        with tc.tile_pool(name="res", bufs=1) as res, \
             tc.tile_pool(name="sb", bufs=2) as sb, \
             tc.tile_pool(name="sb3", bufs=3) as sb3, \
             tc.tile_pool(name="ps", bufs=2, space="PSUM") as ps, \
             tc.tile_pool(name="psl", bufs=2, space="PSUM") as psl:
            bd = res.tile([96, 256], bf16)
            nc.sync.dma_start(out=bd[:], in_=d_bd[:, :])
            ml = res.tile([96, 448], bf16)
            nc.sync.dma_start(out=ml[:], in_=d_ml[:, :])
            invat = res.tile([P, 12], f32)
            nc.sync.dma_start(out=invat[:], in_=d_inva[:, :])
            ftab = res.tile([P, nt * 16], bf16)
            ftabr = res.tile([P, nt * 12], bf16)
            h1o = res.tile([P, nt * 12], f32)
            # ---- local (dst/res) projections: 16 tiles per matmul
            for g in range(ngl):
                xls = sb.tile([96, P], bf16, tag="xls")
                nc.sync.dma_start(out=xls[:], in_=d_xl[:, g * P:(g + 1) * P])
                pl = psl.tile([P, 448], f32, tag="pl")
                nc.tensor.matmul(out=pl[:], lhsT=xls[:], rhs=ml[:],
                                 start=True, stop=True)
                pl3 = pl[:].rearrange("p (t c) -> p t c", t=16, c=28)
                ntile = min(16, nt - g * 16)
                fslice = ftab[:].rearrange("p (t c) -> p t c", t=nt, c=16)[
                    :, g * 16:g * 16 + ntile, :]
                nc.scalar.copy(out=fslice, in_=pl3[:, 0:ntile, 0:16])
                rslice = ftabr[:].rearrange("p (t c) -> p t c", t=nt, c=12)[
                    :, g * 16:g * 16 + ntile, :]
                nc.scalar.copy(out=rslice, in_=pl3[:, 0:ntile, 16:28])
            # ---- supertiles
            for st in range(nst):
                Lp = Lps[st]
                G = Lp // 2
                t0 = st * B
                c0 = int(offs2[st])
                stg = sb.tile([96, G * P], bf16, tag="stg")
                nc.sync.dma_start(out=stg[:], in_=d_pk[:, c0:c0 + G * P])
                Ec = sb.tile([P, Lp * P], bf16, tag="Ec")
                E = sb.tile([P, Lp * P], bf16, tag="E")
                E4 = E[:].rearrange("p (l b c) -> p l b c", l=Lp, b=8, c=16)
                fsl = ftab[:].rearrange("p (t c) -> p t c", t=nt, c=16)[
                    :, t0:t0 + 8, :].rearrange("p b c -> p (b c)")
                for g4 in range(Lp // 4):
                    pt = ps.tile([P, 512], f32, tag="pt")
                    for i in range(2):
                        g = g4 * 2 + i
                        nc.tensor.matmul(out=pt[:, i * 256:(i + 1) * 256],
                                         lhsT=stg[:, g * P:(g + 1) * P],
                                         rhs=bd[:], start=True, stop=True)
                    nc.scalar.copy(out=Ec[:, g4 * 512:(g4 + 1) * 512], in_=pt[:])
                fb = fsl.unsqueeze(1).broadcast_to([P, Lp, P])
                nc.vector.tensor_tensor(
                    out=E[:].rearrange("p (l x) -> p l x", l=Lp, x=P),
                    in0=Ec[:].rearrange("p (l x) -> p l x", l=Lp, x=P),
                    in1=fb, op=AL.add)
                # scores: RP/RN per head (abs reduces over sign-grouped cols)
                RPN = []
                for h in range(2):
                    for pos in (True, False):
                        cw = npos[h] if pos else 5 - npos[h]
                        cb = 2 + 5 * h + (0 if pos else npos[h])
                        r = sb3.tile([P, Lp * 8], bf16, tag=f"r{h}{int(pos)}")
                        if cw > 0:
                            nc.vector.tensor_reduce(
                                out=r[:].rearrange("p (l b) -> p l b", l=Lp, b=8),
                                in_=E4[:, :, :, cb:cb + cw], axis=AX.X, op=AL.add,
                                apply_absolute_value=True)
                        else:
                            nc.vector.memset(r[:], 0.0)
                        RPN.append(r)
                for h in range(2):
                    q = sb3.tile([P, Lp * 8], bf16, tag=f"q{h}")
                    nc.vector.tensor_tensor(out=q[:], in0=RPN[2 * h][:],
                                            in1=RPN[2 * h + 1][:], op=AL.subtract)
                    s = sb3.tile([P, Lp * 8], bf16, tag=f"s{h}")
                    nc.vector.scalar_tensor_tensor(
                        out=s[:].rearrange("p (l b) -> p l b", l=Lp, b=8),
                        in0=E4[:, :, :, 12 + h], scalar=1.5, in1=q[:].rearrange(
                            "p (l b) -> p l b", l=Lp, b=8),
                        op0=AL.mult, op1=AL.add)
                    nc.scalar.activation(
                        out=E4[:, :, :, 12 + h],
                        in_=s[:].rearrange("p (l b) -> p l b", l=Lp, b=8),
                        func=AF.Exp, scale=0.4)
                # a1 scores
                vc = sb3.tile([P, Lp * 8], bf16, tag="vc")
                nc.scalar.copy(out=vc[:].rearrange("p (l b) -> p l b", l=Lp, b=8),
                               in_=E4[:, :, :, 14])
                pa = sb3.tile([P, Lp * 8 * 2], bf16, tag="pa")
                pa3 = pa[:].rearrange("p (l b c) -> p l b c", l=Lp, b=8, c=2)
                nc.scalar.activation(out=pa3, in_=E4[:, :, :, 0:2],
                                     func=AF.Prelu, alpha=NEG_GAT)
                for h in range(2):
                    nc.scalar.activation(out=E4[:, :, :, 14 + h],
                                         in_=pa3[:, :, :, h], func=AF.Exp,
                                         scale=float(sa[h]))
                # mask pads, weight by ex
                vb = vc[:].rearrange("p (l b) -> p l b", l=Lp, b=8
                                     ).unsqueeze(3).broadcast_to([P, Lp, 8, 4])
                nc.vector.tensor_tensor(out=E4[:, :, :, 12:16],
                                        in0=E4[:, :, :, 12:16], in1=vb, op=AL.mult)
                exd = E4[:, :, :, 12:14].unsqueeze(4).broadcast_to([P, Lp, 8, 2, 5])
                wd = E4[:, :, :, 2:12].rearrange("p l b (h f) -> p l b h f", h=2, f=5)
                nc.vector.tensor_tensor(out=wd, in0=wd, in1=exd, op=AL.mult)
                nc.vector.tensor_tensor(out=E4[:, :, :, 0:2], in0=E4[:, :, :, 0:2],
                                        in1=E4[:, :, :, 14:16], op=AL.mult)
                # tree reduce over slots
                e2v = E[:].rearrange("p (l q x) -> p l q x", l=Lp // 2, q=2, x=P)
                T1 = sb3.tile([P, (Lp // 2) * P], bf16, tag="T1")
                t1v = T1[:].rearrange("p (l x) -> p l x", l=Lp // 2, x=P)
                nc.vector.tensor_tensor(out=t1v, in0=e2v[:, :, 0, :],
                                        in1=e2v[:, :, 1, :], op=AL.add)
                t2s = T1[:].rearrange("p (l q x) -> p l q x", l=Lp // 4, q=2, x=P)
                T2 = sb3.tile([P, (Lp // 4) * P], bf16, tag="T2")
                t2v = T2[:].rearrange("p (l x) -> p l x", l=Lp // 4, x=P)
                nc.vector.tensor_tensor(out=t2v, in0=t2s[:, :, 0, :],
                                        in1=t2s[:, :, 1, :], op=AL.add)
                t3s = T2[:].rearrange("p (l q x) -> p l q x", l=Lp // 8, q=2, x=P)
                T3 = sb3.tile([P, (Lp // 8) * P], f32, tag="T3")
                t3v = T3[:].rearrange("p (l x) -> p l x", l=Lp // 8, x=P)
                nc.vector.tensor_tensor(out=t3v, in0=t3s[:, :, 0, :],
                                        in1=t3s[:, :, 1, :], op=AL.add)
                R = sb3.tile([P, P], f32, tag="R")
                nc.vector.tensor_reduce(
                    out=R[:],
                    in_=T3[:].rearrange("p (m x) -> p m x", m=Lp // 8, x=P
                                        ).rearrange("p m x -> p x m"),
                    axis=AX.X, op=AL.add)
                # normalize + residual + elu
                R3 = R[:].rearrange("p (b c) -> p b c", b=8, c=16)
                nc.vector.tensor_scalar_max(out=R3[:, :, 12:16],
                                            in0=R3[:, :, 12:16], scalar1=1e-30)
                REC = sb3.tile([P, 32], f32, tag="REC")
                rec3 = REC[:].rearrange("p (b c) -> p b c", b=8, c=4)
                nc.vector.reciprocal(out=rec3, in_=R3[:, :, 12:16])
                U = sb3.tile([P, 8 * 12], f32, tag="U")
                U3 = U[:].rearrange("p (b c) -> p b c", b=8, c=12)
                u_d = U3[:, :, 2:12].rearrange("p b (h f) -> p b h f", h=2, f=5)
                r_d = R3[:, :, 2:12].rearrange("p b (h f) -> p b h f", h=2, f=5)
                rec_d = rec3[:, :, 0:2].unsqueeze(3).broadcast_to([P, 8, 2, 5])
                nc.vector.tensor_tensor(out=u_d, in0=r_d, in1=rec_d, op=AL.mult)
                nc.vector.tensor_tensor(out=U3[:, :, 0:2], in0=R3[:, :, 0:2],
                                        in1=rec3[:, :, 2:4], op=AL.mult)
                fsl3 = ftab[:].rearrange("p (t c) -> p t c", t=nt, c=16)[
                    :, t0:t0 + 8, 0:12]
                nc.vector.tensor_tensor(out=U3, in0=U3, in1=fsl3, op=AL.subtract)
                ib = invat[:].unsqueeze(1).broadcast_to([P, 8, 12])
                nc.vector.tensor_tensor(out=U3, in0=U3, in1=ib, op=AL.mult)
                rsl3 = ftabr[:].rearrange("p (t c) -> p t c", t=nt, c=12)[
                    :, t0:t0 + 8, :]
                nc.vector.tensor_tensor(out=U3, in0=U3, in1=rsl3, op=AL.add)
                EM = sb3.tile([P, 8 * 12], f32, tag="EM")
                nc.vector.tensor_scalar_min(out=EM[:], in0=U[:], scalar1=0.0)
                nc.scalar.activation(out=EM[:], in_=EM[:], func=AF.Exp)
                nc.vector.tensor_scalar_max(out=U[:], in0=U[:], scalar1=0.0)
                nc.vector.scalar_tensor_tensor(
                    out=h1o[:, t0 * 12:(t0 + 8) * 12], in0=EM[:], scalar=-1.0,
                    in1=U[:], op0=AL.add, op1=AL.add)
            nc.sync.dma_start(out=d_h1o[:, :], in_=h1o[:])
    nc.compile()
    return nc


# ====================================================== launch 1.5 (node proj)
def build_launch15(ncl):
    nch = ncl // 512
    nc = bacc.Bacc("TRN2", target_bir_lowering=False, debug=False, num_devices=NCORE)
    d_h1t = nc.dram_tensor("h1t", [11, ncl], bf16, kind="ExternalInput")
    d_w15 = nc.dram_tensor("w15", [11, 16], bf16, kind="ExternalInput")
    d_p2 = nc.dram_tensor("p2", [16, ncl], bf16, kind="ExternalOutput")
    with TileContext(nc) as tc:
        with tc.tile_pool(name="res", bufs=1) as res, \
             tc.tile_pool(name="sb", bufs=2) as sb, \
             tc.tile_pool(name="ps", bufs=2, space="PSUM") as ps:
            w15 = res.tile([11, 16], bf16)
            nc.sync.dma_start(out=w15[:], in_=d_w15[:, :])
            ot = res.tile([16, ncl], bf16)
            for ch in range(nch):
                ht = sb.tile([11, 512], bf16, tag="ht")
                nc.sync.dma_start(out=ht[:], in_=d_h1t[:, ch * 512:(ch + 1) * 512])
                pp = ps.tile([16, 512], f32, tag="pp")
                nc.tensor.matmul(out=pp[:], lhsT=w15[:], rhs=ht[:],
                                 start=True, stop=True)
                nc.scalar.copy(out=ot[:, ch * 512:(ch + 1) * 512], in_=pp[:])
            nc.sync.dma_start(out=d_p2[:, :], in_=ot[:])
    nc.compile()
    return nc


# =============================================================== launch2 v2
def build_launch2_v2(nst, Lps, soff, nt, npos2):
    s2c = int(soff[-1])
    nmc = nt * P // 512
    nc = bacc.Bacc("TRN2", target_bir_lowering=False, debug=False, num_devices=NCORE)
    d_s2 = nc.dram_tensor("s2", [P, s2c], bf16, kind="ExternalInput")
    d_ft2 = nc.dram_tensor("ft2", [P, nt * 8], bf16, kind="ExternalInput")
    d_fr2 = nc.dram_tensor("fr2", [P, nt * 4], bf16, kind="ExternalInput")
    d_inva = nc.dram_tensor("inva2", [P, 4], f32, kind="ExternalInput")
    d_m14 = nc.dram_tensor("m14b", [P, nt * 14], bf16, kind="ExternalInput")
    d_w1 = nc.dram_tensor("w1", [14, 196], bf16, kind="ExternalInput")
    d_w2 = nc.dram_tensor("w2", [196, 196], bf16, kind="ExternalInput")
    d_w3 = nc.dram_tensor("w3", [196, 14], bf16, kind="ExternalInput")
    d_w4 = nc.dram_tensor("w4", [14, 1], bf16, kind="ExternalInput")
    d_b1 = nc.dram_tensor("b1", [196], f32, kind="ExternalInput")
    d_b2 = nc.dram_tensor("b2", [196], f32, kind="ExternalInput")
    d_b3 = nc.dram_tensor("b3", [14], f32, kind="ExternalInput")
    d_b4 = nc.dram_tensor("b4", [1], f32, kind="ExternalInput")
    d_out = nc.dram_tensor("out", [nmc, 512], f32, kind="ExternalOutput")
    with TileContext(nc) as tc, nc.allow_low_precision("bf16 GAT partials"):
        with tc.tile_pool(name="res", bufs=1) as res, \
             tc.tile_pool(name="sb", bufs=2) as sb, \
             tc.tile_pool(name="sb3", bufs=3) as sb3, \
             tc.tile_pool(name="psl", bufs=2, space="PSUM") as psl, \
             tc.tile_pool(name="psm", bufs=4, space="PSUM") as psm:
            ident = res.tile([P, P], bf16)
            make_identity(nc, ident[:])
            ft2 = res.tile([P, nt * 8], bf16)
            nc.sync.dma_start(out=ft2[:], in_=d_ft2[:, :])
            fr2 = res.tile([P, nt * 4], bf16)
            nc.sync.dma_start(out=fr2[:], in_=d_fr2[:, :])
            invat = res.tile([P, 4], f32)
            nc.sync.dma_start(out=invat[:], in_=d_inva[:, :])
            m14 = res.tile([P, nt * 14], bf16)
            nc.sync.dma_start(out=m14[:], in_=d_m14[:, :])
            w1 = res.tile([14, 196], bf16, tag="w1")
            nc.sync.dma_start(out=w1[:], in_=d_w1[:, :])
            w2a = res.tile([P, 196], bf16, tag="w2a")
            nc.sync.dma_start(out=w2a[:], in_=d_w2[0:128, :])
            w2b = res.tile([68, 196], bf16, tag="w2b")
            nc.sync.dma_start(out=w2b[:], in_=d_w2[128:196, :])
            w3a = res.tile([P, 14], bf16, tag="w3a")
            nc.sync.dma_start(out=w3a[:], in_=d_w3[0:128, :])
            w3b = res.tile([68, 14], bf16, tag="w3b")
            nc.sync.dma_start(out=w3b[:], in_=d_w3[128:196, :])
            w4 = res.tile([14, 1], bf16, tag="w4")
            nc.sync.dma_start(out=w4[:], in_=d_w4[:, :])
            b1ca = res.tile([P, 1], f32, tag="b1ca")
            nc.sync.dma_start(out=b1ca[:], in_=d_b1[0:128, None])
            b1cb = res.tile([68, 1], f32, tag="b1cb")
            nc.sync.dma_start(out=b1cb[:], in_=d_b1[128:196, None])
            b2ca = res.tile([P, 1], f32, tag="b2ca")
            nc.sync.dma_start(out=b2ca[:], in_=d_b2[0:128, None])
            b2cb = res.tile([68, 1], f32, tag="b2cb")
            nc.sync.dma_start(out=b2cb[:], in_=d_b2[128:196, None])
            b3c = res.tile([14, 1], f32, tag="b3c")
            nc.sync.dma_start(out=b3c[:], in_=d_b3[:, None])
            b4c = res.tile([1, 1], f32, tag="b4c")
            nc.sync.dma_start(out=b4c[:], in_=d_b4[:, None])
            for st in range(nst):
                Lp = Lps[st]
                t0 = st * B
                c0 = int(soff[st])
                s2t = sb.tile([P, 64 * Lp], bf16, tag="s2t")
                nc.sync.dma_start(out=s2t[:], in_=d_s2[:, c0:c0 + 64 * Lp])
                E = sb.tile([P, 64 * Lp], bf16, tag="E2")
                E4 = E[:].rearrange("p (l b c) -> p l b c", l=Lp, b=8, c=8)
                fsl = ft2[:].rearrange("p (t c) -> p t c", t=nt, c=8)[
                    :, t0:t0 + 8, :].rearrange("p b c -> p (b c)")
                fb = fsl.unsqueeze(1).broadcast_to([P, Lp, 64])
                nc.vector.tensor_tensor(
                    out=E[:].rearrange("p (l x) -> p l x", l=Lp, x=64),
                    in0=s2t[:].rearrange("p (l x) -> p l x", l=Lp, x=64),
                    in1=fb, op=AL.add)
                RPN = []
                for h in range(2):
                    for pos in (True, False):
                        cw = npos2[h] if pos else 2 - npos2[h]
                        cb = 2 * h + (0 if pos else npos2[h])
                        r = sb3.tile([P, Lp * 8], bf16, tag=f"r2{h}{int(pos)}")
                        if cw > 0:
                            nc.vector.tensor_reduce(
                                out=r[:].rearrange("p (l b) -> p l b", l=Lp, b=8),
                                in_=E4[:, :, :, cb:cb + cw], axis=AX.X, op=AL.add,
                                apply_absolute_value=True)
                        else:
                            nc.vector.memset(r[:], 0.0)
                        RPN.append(r)
                EXC = sb3.tile([P, 2 * Lp * 8], bf16, tag="EXC")
                for h in range(2):
                    q = sb3.tile([P, Lp * 8], bf16, tag=f"q2{h}")
                    nc.vector.tensor_tensor(out=q[:], in0=RPN[2 * h][:],
                                            in1=RPN[2 * h + 1][:], op=AL.subtract)
                    s = sb3.tile([P, Lp * 8], bf16, tag=f"s2{h}")
                    nc.vector.scalar_tensor_tensor(
                        out=s[:].rearrange("p (l b) -> p l b", l=Lp, b=8),
                        in0=E4[:, :, :, 4 + h], scalar=1.5,
                        in1=q[:].rearrange("p (l b) -> p l b", l=Lp, b=8),
                        op0=AL.mult, op1=AL.add)
                    nc.scalar.activation(out=EXC[:, h * Lp * 8:(h + 1) * Lp * 8],
                                         in_=s[:], func=AF.Exp, scale=0.4)
                exv = EXC[:].rearrange("p (j l b) -> p j l b", j=2, l=Lp, b=8
                                       ).rearrange("p j l b -> p l b j")
                vbc = E4[:, :, :, 6:7].broadcast_to([P, Lp, 8, 2])
                nc.vector.tensor_tensor(out=E4[:, :, :, 4:6], in0=exv, in1=vbc,
                                        op=AL.mult)
                exd = E4[:, :, :, 4:6].unsqueeze(4).broadcast_to([P, Lp, 8, 2, 2])
                wd = E4[:, :, :, 0:4].rearrange("p l b (h f) -> p l b h f", h=2, f=2)
                nc.vector.tensor_tensor(out=wd, in0=wd, in1=exd, op=AL.mult)
                e2v = E[:].rearrange("p (l q x) -> p l q x", l=Lp // 2, q=2, x=64)
                T1 = sb3.tile([P, (Lp // 2) * 64], bf16, tag="T1b")
                t1v = T1[:].rearrange("p (l x) -> p l x", l=Lp // 2, x=64)
                nc.vector.tensor_tensor(out=t1v, in0=e2v[:, :, 0, :],
                                        in1=e2v[:, :, 1, :], op=AL.add)
                t2s = T1[:].rearrange("p (l q x) -> p l q x", l=Lp // 4, q=2, x=64)
                T2 = sb3.tile([P, (Lp // 4) * 64], bf16, tag="T2b")
                t2v = T2[:].rearrange("p (l x) -> p l x", l=Lp // 4, x=64)
                nc.vector.tensor_tensor(out=t2v, in0=t2s[:, :, 0, :],
                                        in1=t2s[:, :, 1, :], op=AL.add)
                t3s = T2[:].rearrange("p (l q x) -> p l q x", l=Lp // 8, q=2, x=64)
                T3 = sb3.tile([P, (Lp // 8) * 64], f32, tag="T3b")
                t3v = T3[:].rearrange("p (l x) -> p l x", l=Lp // 8, x=64)
                nc.vector.tensor_tensor(out=t3v, in0=t3s[:, :, 0, :],
                                        in1=t3s[:, :, 1, :], op=AL.add)
                R = sb3.tile([P, 64], f32, tag="Rb")
                t3r = T3[:].rearrange("p (m x) -> p m x", m=Lp // 8, x=64)
                nc.vector.tensor_reduce(
                    out=R[:], in_=t3r.rearrange("p m x -> p x m"),
                    axis=AX.X, op=AL.add)
                R3 = R[:].rearrange("p (b c) -> p b c", b=8, c=8)
                nc.vector.tensor_scalar_max(out=R3[:, :, 4:6],
                                            in0=R3[:, :, 4:6], scalar1=1e-30)
                REC = sb3.tile([P, 16], f32, tag="RECb")
                rec3 = REC[:].rearrange("p (b c) -> p b c", b=8, c=2)
                nc.vector.reciprocal(out=rec3, in_=R3[:, :, 4:6])
                U = sb3.tile([P, 8 * 4], f32, tag="Ub")
                U3 = U[:].rearrange("p (b c) -> p b c", b=8, c=4)
                u_d = U3.rearrange("p b (h f) -> p b h f", h=2, f=2)
                r_d = R3[:, :, 0:4].rearrange("p b (h f) -> p b h f", h=2, f=2)
                rec_d = rec3.unsqueeze(3).broadcast_to([P, 8, 2, 2])
                nc.vector.tensor_tensor(out=u_d, in0=r_d, in1=rec_d, op=AL.mult)
                fsl3 = ft2[:].rearrange("p (t c) -> p t c", t=nt, c=8)[
                    :, t0:t0 + 8, 0:4]
                nc.vector.tensor_tensor(out=U3, in0=U3, in1=fsl3, op=AL.subtract)
                ib = invat[:].unsqueeze(1).broadcast_to([P, 8, 4])
                nc.vector.tensor_tensor(out=U3, in0=U3, in1=ib, op=AL.mult)
                rsl3 = fr2[:].rearrange("p (t c) -> p t c", t=nt, c=4)[
                    :, t0:t0 + 8, :]
                nc.vector.tensor_tensor(out=U3, in0=U3, in1=rsl3, op=AL.add)
                EM = sb3.tile([P, 8 * 4], f32, tag="EMb")
                nc.vector.tensor_scalar_min(out=EM[:], in0=U[:], scalar1=0.0)
                nc.scalar.activation(out=EM[:], in_=EM[:], func=AF.Exp)
                nc.vector.tensor_scalar_max(out=U[:], in0=U[:], scalar1=0.0)
                m14s = m14[:].rearrange("p (t c) -> p t c", t=nt, c=14)[
                    :, t0:t0 + 8, 2:6]
                nc.vector.scalar_tensor_tensor(
                    out=m14s, in0=EM[:].rearrange("p (b c) -> p b c", b=8, c=4),
                    scalar=-1.0,
                    in1=U3, op0=AL.add, op1=AL.add)
                # ---- MLP for this supertile's two 512-node chunks
                for half in range(2):
                    mc = st * 2 + half
                    tm0 = t0 + half * 4
                    r0 = sb.tile([14, 512], bf16, tag="r0")
                    for b in range(4):
                        ptm = psm.tile([14, P], bf16, tag="ptm")
                        nc.tensor.transpose(
                            out=ptm[:], in_=m14[:, (tm0 + b) * 14:(tm0 + b + 1) * 14],
                            identity=ident[:])
                        nc.vector.tensor_copy(out=r0[:, b * P:(b + 1) * P], in_=ptm[:])
                    p1a = psl.tile([P, 512], f32, tag="pA")
                    nc.tensor.matmul(out=p1a[:], lhsT=w1[:, 0:128], rhs=r0[:], start=True, stop=True)
                    p1b = psl.tile([68, 512], f32, tag="pB")
                    nc.tensor.matmul(out=p1b[:], lhsT=w1[:, 128:196], rhs=r0[:], start=True, stop=True)
                    r1a = sb.tile([P, 512], bf16, tag="r1a")
                    nc.scalar.activation(out=r1a[:], in_=p1a[:], func=AF.Prelu,
                                         alpha=NEG_MLP, bias=b1ca[:])
                    r1b = sb.tile([68, 512], bf16, tag="r1b")
                    nc.scalar.activation(out=r1b[:], in_=p1b[:], func=AF.Prelu,
                                         alpha=NEG_MLP, bias=b1cb[:])
                    p2a = psl.tile([P, 512], f32, tag="pA")
                    nc.tensor.matmul(out=p2a[:], lhsT=w2a[:, 0:128], rhs=r1a[:], start=True, stop=False)
                    nc.tensor.matmul(out=p2a[:], lhsT=w2b[:, 0:128], rhs=r1b[:], start=False, stop=True)
                    p2b = psl.tile([68, 512], f32, tag="pB")
                    nc.tensor.matmul(out=p2b[:], lhsT=w2a[:, 128:196], rhs=r1a[:], start=True, stop=False)
                    nc.tensor.matmul(out=p2b[:], lhsT=w2b[:, 128:196], rhs=r1b[:], start=False, stop=True)
                    r2a = sb.tile([P, 512], bf16, tag="r2a")
                    nc.scalar.activation(out=r2a[:], in_=p2a[:], func=AF.Prelu,
                                         alpha=NEG_MLP, bias=b2ca[:])
                    r2b = sb.tile([68, 512], bf16, tag="r2b")
                    nc.scalar.activation(out=r2b[:], in_=p2b[:], func=AF.Prelu,
                                         alpha=NEG_MLP, bias=b2cb[:])
                    p3 = psl.tile([14, 512], f32, tag="pA")
                    nc.tensor.matmul(out=p3[:], lhsT=w3a[:], rhs=r2a[:], start=True, stop=False)
                    nc.tensor.matmul(out=p3[:], lhsT=w3b[:], rhs=r2b[:], start=False, stop=True)
                    r3 = sb.tile([14, 512], bf16, tag="r3")
                    nc.scalar.activation(out=r3[:], in_=p3[:], func=AF.Prelu,
                                         alpha=NEG_MLP, bias=b3c[:])
                    po = psl.tile([1, 512], f32, tag="pB")
                    nc.tensor.matmul(out=po[:], lhsT=w4[:], rhs=r3[:], start=True, stop=True)
                    sg = sb.tile([1, 512], f32, tag="sg")
                    nc.scalar.activation(out=sg[:], in_=po[:], func=AF.Sigmoid, bias=b4c[:])
                    nc.sync.dma_start(out=d_out[mc:mc + 1, :], in_=sg[:])
    nc.compile()
    return nc


# =============================================================== launch2 (v1)
def build_launch2(nst, Ls, offs, nt):
    nchunk = nt // 8
    totc = int(offs[-1])
    nmc = nt * P // 512
    nc = bacc.Bacc("TRN2", target_bir_lowering=False, debug=False, num_devices=NCORE)
    d_pk = nc.dram_tensor("hde", [88, totc], bf16, kind="ExternalInput")
    d_lpk = nc.dram_tensor("h1l", [88, nchunk * P], bf16, kind="ExternalInput")
    d_np = nc.dram_tensor("npad", [P, nt], f32, kind="ExternalInput")
    d_at = nc.dram_tensor("attn4", [P, 4], bf16, kind="ExternalInput")
    d_b2e = nc.dram_tensor("bd2e", [88, 64], bf16, kind="ExternalInput")
    d_b2l = nc.dram_tensor("bd2l", [88, 64], bf16, kind="ExternalInput")
    d_hatt = nc.dram_tensor("hatt", [P, nt * 2], bf16, kind="ExternalInput")
    d_x = nc.dram_tensor("xpm", [P, nt * 8], bf16, kind="ExternalInput")
    d_w1 = nc.dram_tensor("w1", [14, 196], bf16, kind="ExternalInput")
    d_w2 = nc.dram_tensor("w2", [196, 196], bf16, kind="ExternalInput")
    d_w3 = nc.dram_tensor("w3", [196, 14], bf16, kind="ExternalInput")
    d_w4 = nc.dram_tensor("w4", [14, 1], bf16, kind="ExternalInput")
    d_b1 = nc.dram_tensor("b1", [196], f32, kind="ExternalInput")
    d_b2 = nc.dram_tensor("b2", [196], f32, kind="ExternalInput")
    d_b3 = nc.dram_tensor("b3", [14], f32, kind="ExternalInput")
    d_b4 = nc.dram_tensor("b4", [1], f32, kind="ExternalInput")
    d_out = nc.dram_tensor("out", [nmc, 512], f32, kind="ExternalOutput")
    with TileContext(nc) as tc:
        with tc.tile_pool(name="res", bufs=1) as res, \
             tc.tile_pool(name="sb", bufs=2) as sb, \
             tc.tile_pool(name="sb3", bufs=3) as sb3:
            ident = res.tile([P, P], bf16)
            make_identity(nc, ident[:])
            attn = res.tile([P, 4], bf16)
            nc.sync.dma_start(out=attn[:], in_=d_at[:, :])
            npad_t = res.tile([P, nt], f32)
            nc.sync.dma_start(out=npad_t[:], in_=d_np[:, :])
            b2e = res.tile([88, 64], bf16, tag="b2e")
            nc.sync.dma_start(out=b2e[:], in_=d_b2e[:, :])
            b2l = res.tile([88, 64], bf16, tag="b2l")
            nc.sync.dma_start(out=b2l[:], in_=d_b2l[:, :])
            hatt = res.tile([P, nt * 2], bf16)
            nc.sync.dma_start(out=hatt[:], in_=d_hatt[:, :])
            xpm = res.tile([P, nt * 8], bf16)
            nc.sync.dma_start(out=xpm[:], in_=d_x[:, :])
            ftab2 = res.tile([P, nt * 4], bf16)
            ftab2r = res.tile([P, nt * 4], f32)
            h2o = res.tile([P, nt * 4], f32)
            w1 = res.tile([14, 196], bf16, tag="w1")
            nc.sync.dma_start(out=w1[:], in_=d_w1[:, :])
            w2a = res.tile([P, 196], bf16, tag="w2a")
            nc.sync.dma_start(out=w2a[:], in_=d_w2[0:128, :])
            w2b = res.tile([68, 196], bf16, tag="w2b")
            nc.sync.dma_start(out=w2b[:], in_=d_w2[128:196, :])
            w3a = res.tile([P, 14], bf16, tag="w3a")
            nc.sync.dma_start(out=w3a[:], in_=d_w3[0:128, :])
            w3b = res.tile([68, 14], bf16, tag="w3b")
            nc.sync.dma_start(out=w3b[:], in_=d_w3[128:196, :])
            w4 = res.tile([14, 1], bf16, tag="w4")
            nc.sync.dma_start(out=w4[:], in_=d_w4[:, :])
            b1ca = res.tile([P, 1], f32, tag="b1ca")
            nc.sync.dma_start(out=b1ca[:], in_=d_b1[0:128, None])
            b1cb = res.tile([68, 1], f32, tag="b1cb")
            nc.sync.dma_start(out=b1cb[:], in_=d_b1[128:196, None])
            b2ca = res.tile([P, 1], f32, tag="b2ca")
            nc.sync.dma_start(out=b2ca[:], in_=d_b2[0:128, None])
            b2cb = res.tile([68, 1], f32, tag="b2cb")
            nc.sync.dma_start(out=b2cb[:], in_=d_b2[128:196, None])
            b3c = res.tile([14, 1], f32, tag="b3c")
            nc.sync.dma_start(out=b3c[:], in_=d_b3[:, None])
            b4c = res.tile([1, 1], f32, tag="b4c")
            nc.sync.dma_start(out=b4c[:], in_=d_b4[:, None])
            with tc.tile_pool(name="ps", bufs=2, space="PSUM") as ps, \
                 tc.tile_pool(name="psl", bufs=1, space="PSUM") as psl:
                for ch in range(nchunk):
                    stg = sb.tile([88, P], bf16, tag="lstg")
                    nc.sync.dma_start(out=stg[:], in_=d_lpk[:, ch * P:(ch + 1) * P])
                    pmm = psl.tile([64, P], f32, tag="lmm")
                    nc.tensor.matmul(out=pmm[:], lhsT=b2l[:], rhs=stg[:],
                                     start=True, stop=True)
                    cp = sb.tile([64, P], bf16, tag="lcp")
                    nc.vector.tensor_copy(out=cp[:], in_=pmm[:])
                    pt = psl.tile([P, 64], bf16, tag="ltt")
                    nc.tensor.transpose(out=pt[:], in_=cp[:], identity=ident[0:64, 0:64])
                    psl_ap = pt[:].rearrange("p (a c) -> p a c", a=8, c=8)
                    f2s = ftab2[:].rearrange("p (t c) -> p t c", t=nt, c=4)[:, ch * 8:(ch + 1) * 8, :]
                    nc.scalar.copy(out=f2s, in_=psl_ap[:, :, 0:4])
                    f2r = ftab2r[:].rearrange("p (t c) -> p t c", t=nt, c=4)[:, ch * 8:(ch + 1) * 8, :]
                    nc.scalar.copy(out=f2r, in_=psl_ap[:, :, 4:8])
                for st in range(nst):
                    L = Ls[st]
                    ncols = B * L * 16
                    stg = sb.tile([88, ncols], bf16, tag="estg")
                    nc.sync.dma_start(out=stg[:], in_=d_pk[:, int(offs[st]):int(offs[st]) + ncols])
                    G = sb.tile([P, ncols // 2], bf16, tag="G")
                    emit_project(nc, sb, ps, stg[:], b2e[:], ident[:], G, ncols, gcopy="vec", sp=8)
                    t0 = st * B
                    fd_ap = ftab2[:].rearrange("p (t c) -> p t c", t=nt, c=4)[:, t0:t0 + B, :]
                    emit_gat_supertile(nc, sb, sb3, G, L, 4, 2, fd_ap,
                                       ftab2r[:, t0 * 4:(t0 + B) * 4],
                                       npad_t[:, t0:t0 + B], attn[:],
                                       0, 2, False, h2o[:, t0 * 4:(t0 + B) * 4], gsp=8)
                for mc in range(nmc):
                    t0 = mc * 4
                    m14 = sb.tile([P, 4 * 14], bf16, tag="m14")
                    m143 = m14[:].rearrange("p (t c) -> p t c", t=4, c=14)
                    ha = hatt[:].rearrange("p (t c) -> p t c", t=nt, c=2)[:, t0:t0 + 4, :]
                    nc.vector.tensor_copy(out=m143[:, :, 0:2], in_=ha)
                    h2s = h2o[:].rearrange("p (t c) -> p t c", t=nt, c=4)[:, t0:t0 + 4, :]
                    nc.vector.tensor_copy(out=m143[:, :, 2:6], in_=h2s)
                    xs = xpm[:].rearrange("p (t c) -> p t c", t=nt, c=8)[:, t0:t0 + 4, :]
                    nc.vector.tensor_copy(out=m143[:, :, 6:14], in_=xs)
                    r0 = sb.tile([14, 512], bf16, tag="r0")
                    for b in range(4):
                        ptm = psl.tile([14, P], bf16, tag="ltt")
                        nc.tensor.transpose(out=ptm[:], in_=m14[:, b * 14:(b + 1) * 14],
                                            identity=ident[:])
                        nc.vector.tensor_copy(out=r0[:, b * P:(b + 1) * P], in_=ptm[:])
                    p1a = psl.tile([P, 512], f32, tag="pA")
                    nc.tensor.matmul(out=p1a[:], lhsT=w1[:, 0:128], rhs=r0[:], start=True, stop=True)
                    p1b = psl.tile([68, 512], f32, tag="pB")
                    nc.tensor.matmul(out=p1b[:], lhsT=w1[:, 128:196], rhs=r0[:], start=True, stop=True)
                    r1a = sb.tile([P, 512], bf16, tag="r1a")
                    nc.scalar.activation(out=r1a[:], in_=p1a[:], func=AF.Prelu,
                                         alpha=NEG_MLP, bias=b1ca[:])
                    r1b = sb.tile([68, 512], bf16, tag="r1b")
                    nc.scalar.activation(out=r1b[:], in_=p1b[:], func=AF.Prelu,
                                         alpha=NEG_MLP, bias=b1cb[:])
                    p2a = psl.tile([P, 512], f32, tag="pA")
                    nc.tensor.matmul(out=p2a[:], lhsT=w2a[:, 0:128], rhs=r1a[:], start=True, stop=False)
                    nc.tensor.matmul(out=p2a[:], lhsT=w2b[:, 0:128], rhs=r1b[:], start=False, stop=True)
                    p2b = psl.tile([68, 512], f32, tag="pB")
                    nc.tensor.matmul(out=p2b[:], lhsT=w2a[:, 128:196], rhs=r1a[:], start=True, stop=False)
                    nc.tensor.matmul(out=p2b[:], lhsT=w2b[:, 128:196], rhs=r1b[:], start=False, stop=True)
                    r2a = sb.tile([P, 512], bf16, tag="r2a")
                    nc.scalar.activation(out=r2a[:], in_=p2a[:], func=AF.Prelu,
                                         alpha=NEG_MLP, bias=b2ca[:])
                    r2b = sb.tile([68, 512], bf16, tag="r2b")
                    nc.scalar.activation(out=r2b[:], in_=p2b[:], func=AF.Prelu,
                                         alpha=NEG_MLP, bias=b2cb[:])
                    p3 = psl.tile([14, 512], f32, tag="pA")
                    nc.tensor.matmul(out=p3[:], lhsT=w3a[:], rhs=r2a[:], start=True, stop=False)
                    nc.tensor.matmul(out=p3[:], lhsT=w3b[:], rhs=r2b[:], start=False, stop=True)
                    r3 = sb.tile([14, 512], bf16, tag="r3")
                    nc.scalar.activation(out=r3[:], in_=p3[:], func=AF.Prelu,
                                         alpha=NEG_MLP, bias=b3c[:])
                    po = psl.tile([1, 512], f32, tag="pB")
                    nc.tensor.matmul(out=po[:], lhsT=w4[:], rhs=r3[:], start=True, stop=True)
                    sg = sb.tile([1, 512], f32, tag="sg")
                    nc.scalar.activation(out=sg[:], in_=po[:], func=AF.Sigmoid, bias=b4c[:])
                    nc.sync.dma_start(out=d_out[mc:mc + 1, :], in_=sg[:])
    nc.compile()
    return nc


# ---- v1 helpers still used by launch2
def emit_project(nc, sb, ps, stg, bd_t, ident, G, ncols, gcopy="act", sp=16):
    M = 8 * sp
    for ci, c0 in enumerate(range(0, ncols, 512)):
        cw = min(512, ncols - c0)
        pmm = ps.tile([M, 512], f32, tag="mmout")
        nc.tensor.matmul(out=pmm[:, :cw], lhsT=bd_t, rhs=stg[:, c0:c0 + cw],
                         start=True, stop=True)
        cp = sb.tile([M, 512], bf16, tag="mmcopy")
        if ci % 2 == 0:
            nc.vector.tensor_copy(out=cp[:, :cw], in_=pmm[:, :cw])
        else:
            nc.scalar.copy(out=cp[:, :cw], in_=pmm[:, :cw])
        for k0 in range(0, cw, P):
            pt = ps.tile([P, M], bf16, tag="tout")
            nc.tensor.transpose(out=pt[:], in_=cp[:, k0:k0 + P], identity=ident[0:M, 0:M])
            q0 = (c0 + k0) // P * 8
            if gcopy == "act":
                nc.scalar.copy(out=G[:, q0 * sp:(q0 + 8) * sp], in_=pt[:])
            else:
                nc.vector.tensor_copy(out=G[:, q0 * sp:(q0 + 8) * sp], in_=pt[:])


def emit_gat_supertile(nc, sb, sb3, G, L, C, NH, fd_ap, ftr_ap, npad_ap, attn_ap,
                       d_off, d_F, a1, out_ap, gsp=16):
    BL = B * L
    nd = 2 if a1 else 0
    nrh = NH - nd
    G4 = G[:].rearrange("p (b l c) -> p b l c", b=B, l=L, c=gsp)[:, :, :, 0:C]
    fdb = fd_ap.unsqueeze(2).broadcast_to([P, B, L, C])
    e = sb.tile([P, BL * C], bf16, tag="bA")
    e4 = e[:].rearrange("p (b l c) -> p b l c", b=B, l=L, c=C)
    z = sb.tile([P, BL * C], bf16, tag="bB")
    z4 = z[:].rearrange("p (b l c) -> p b l c", b=B, l=L, c=C)
    zA = sb.tile([P, BL * C], bf16, tag="bC")
    zA4 = zA[:].rearrange("p (b l c) -> p b l c", b=B, l=L, c=C)
    atb = attn_ap.unsqueeze(1).unsqueeze(1).broadcast_to([P, B, L, C])
    ex = sb3.tile([P, BL * NH], bf16, tag="ex")
    ex4 = ex[:].rearrange("p (b l h) -> p b l h", b=B, l=L, h=NH)
    sd = sb3.tile([P, BL * nrh], f32, tag="sd")
    sd4 = sd[:].rearrange("p (b l h) -> p b l h", b=B, l=L, h=nrh)
    hb = B // 2
    for hf in range(2):
        b0, b1 = hf * hb, (hf + 1) * hb
        nc.vector.tensor_tensor(out=e4[:, b0:b1], in0=G4[:, b0:b1],
                                in1=fdb[:, b0:b1], op=AL.add)
        nc.scalar.activation(out=z4[:, b0:b1], in_=e4[:, b0:b1],
                             func=AF.Prelu, alpha=NEG_GAT)
        nc.vector.tensor_tensor(out=zA4[:, b0:b1], in0=z4[:, b0:b1],
                                in1=atb[:, b0:b1], op=AL.mult)
        for h in range(nrh):
            nc.vector.tensor_reduce(out=sd4[:, b0:b1, :, h],
                                    in_=zA4[:, b0:b1, :, d_off + h::2],
                                    axis=AX.X, op=AL.add)
        if a1:
            nc.scalar.activation(out=ex4[:, b0:b1, :, 0:2],
                                 in_=zA4[:, b0:b1, :, 0:2], func=AF.Exp)
        nc.scalar.activation(out=ex4[:, b0:b1, :, nd:NH],
                             in_=sd4[:, b0:b1], func=AF.Exp)
    den = sb3.tile([P, B * NH], f32, tag="den")
    den3 = den[:].rearrange("p (b h) -> p b h", b=B, h=NH)
    exT = ex4.rearrange("p b l h -> p b h l")
    nc.vector.tensor_reduce(out=den3, in_=exT, axis=AX.X, op=AL.add)
    w = sb.tile([P, BL * C], bf16, tag="bB")
    w4 = w[:].rearrange("p (b l c) -> p b l c", b=B, l=L, c=C)
    if a1:
        nc.vector.tensor_tensor(out=w4[:, :, :, 0:2], in0=G4[:, :, :, 0:2],
                                in1=ex4[:, :, :, 0:2], op=AL.mult)
    exd = ex4[:, :, :, nd:NH].unsqueeze(3).broadcast_to([P, B, L, d_F, nrh])
    G5 = G4[:, :, :, d_off:C].rearrange("p b l (f h) -> p b l f h", f=d_F, h=nrh)
    w5 = w4[:, :, :, d_off:C].rearrange("p b l (f h) -> p b l f h", f=d_F, h=nrh)
    nc.vector.tensor_tensor(out=w5, in0=G5, in1=exd, op=AL.mult)
    zp = sb3.tile([P, B * C], bf16, tag="zp")
    zp3 = zp[:].rearrange("p (b c) -> p b c", b=B, c=C)
    nc.scalar.activation(out=zp3, in_=fd_ap, func=AF.Prelu, alpha=NEG_GAT)
    zpA = sb3.tile([P, B * C], bf16, tag="zpA")
    zpA3 = zpA[:].rearrange("p (b c) -> p b c", b=B, c=C)
    atb2 = attn_ap.unsqueeze(1).broadcast_to([P, B, C])
    nc.vector.tensor_tensor(out=zpA3, in0=zp3, in1=atb2, op=AL.mult)
    exp_p = sb3.tile([P, B * NH], bf16, tag="exp_p")
    exp_p3 = exp_p[:].rearrange("p (b h) -> p b h", b=B, h=NH)
    spd = sb3.tile([P, B * nrh], f32, tag="spd")
    spd3 = spd[:].rearrange("p (b h) -> p b h", b=B, h=nrh)
    for h in range(nrh):
        nc.vector.tensor_reduce(out=spd3[:, :, h], in_=zpA3[:, :, d_off + h::2],
                                axis=AX.X, op=AL.add)
    if a1:
        nc.scalar.activation(out=exp_p3[:, :, 0:2], in_=zpA3[:, :, 0:2], func=AF.Exp)
    nc.scalar.activation(out=exp_p3[:, :, nd:NH], in_=spd3, func=AF.Exp)
    padm = sb3.tile([P, B * NH], f32, tag="padm")
    padm3 = padm[:].rearrange("p (b h) -> p b h", b=B, h=NH)
    npb = npad_ap.unsqueeze(2).broadcast_to([P, B, NH])
    nc.vector.tensor_tensor(out=padm3, in0=exp_p3, in1=npb, op=AL.mult)
    nc.vector.tensor_tensor(out=den[:], in0=den[:], in1=padm[:], op=AL.subtract)
    nc.vector.tensor_scalar_max(out=den[:], in0=den[:], scalar1=1e-30)
    rec = sb3.tile([P, B * NH], f32, tag="rec")
    nc.vector.reciprocal(out=rec[:], in_=den[:])
    rec3 = rec[:].rearrange("p (b h) -> p b h", b=B, h=NH)
    rstn = sb3.tile([P, B * C], f32, tag="rstn")
    rstn3 = rstn[:].rearrange("p (b c) -> p b c", b=B, c=C)
    wT = w4.rearrange("p b l c -> p b c l")
    nc.vector.tensor_reduce(out=rstn3, in_=wT, axis=AX.X, op=AL.add)
    if a1:
        nc.vector.tensor_tensor(out=rstn3[:, :, 0:2], in0=rstn3[:, :, 0:2],
                                in1=rec3[:, :, 0:2], op=AL.mult)
        rr4 = rec3[:, :, nd:NH].unsqueeze(2).broadcast_to([P, B, d_F, nrh])
        rd4 = rstn3[:, :, d_off:C].rearrange("p b (f h) -> p b f h", f=d_F, h=nrh)
        nc.vector.tensor_tensor(out=rd4, in0=rd4, in1=rr4, op=AL.mult)
    else:
        rr4 = rec3.unsqueeze(2).broadcast_to([P, B, d_F, nrh])
        rd4 = rstn3.rearrange("p b (f h) -> p b f h", f=d_F, h=nrh)
        nc.vector.tensor_tensor(out=rd4, in0=rd4, in1=rr4, op=AL.mult)
    nc.vector.tensor_tensor(out=rstn[:], in0=rstn[:], in1=ftr_ap, op=AL.add)
    tmin = sb3.tile([P, B * C], f32, tag="tmin")
    nc.vector.tensor_scalar_min(out=tmin[:], in0=rstn[:], scalar1=0.0)
    epx = sb3.tile([P, B * C], f32, tag="epx")
    nc.scalar.activation(out=epx[:], in_=tmin[:], func=AF.Exp)
    nc.vector.tensor_scalar_max(out=rstn[:], in0=rstn[:], scalar1=0.0)
    nc.vector.tensor_tensor(out=rstn[:], in0=rstn[:], in1=epx[:], op=AL.add)
    nc.vector.tensor_scalar_add(out=out_ap, in0=rstn[:], scalar1=-1.0)


# ============================================================ l2 v2 weights
def build_l2_weights(inputs):
    Ws = np.asarray(inputs['d2_Wsrc'], np.float64)   # [10,4] col 2h+f
    bs = np.asarray(inputs['d2_bsrc'], np.float64)
    Wd = np.asarray(inputs['d2_Wdst'], np.float64)
    bd = np.asarray(inputs['d2_bdst'], np.float64)
    at = np.asarray(inputs['d2_attn'], np.float64)   # [2,2]
    Wr = np.asarray(inputs['d2_Wres'], np.float64)
    br = np.asarray(inputs['d2_bias'], np.float64)
    perm2, npos2 = [], []
    for h in range(2):
        fpos = [f for f in range(2) if at[h, f] > 0]
        fneg = [f for f in range(2) if at[h, f] <= 0]
        perm2.append(fpos + fneg)
        npos2.append(len(fpos))
    aa = np.abs(at)
    aa[aa == 0] = 1e-12
    w15 = np.zeros((11, 16), np.float64)
    for h in range(2):
        for j, f in enumerate(perm2[h]):
            w15[0:10, 2 * h + j] = aa[h, f] * Ws[:, 2 * h + f]
            w15[10, 2 * h + j] = aa[h, f] * bs[2 * h + f]
            w15[0:10, 6 + 2 * h + j] = aa[h, f] * Wd[:, 2 * h + f]
            w15[10, 6 + 2 * h + j] = aa[h, f] * bd[2 * h + f]
            w15[0:10, 12 + 2 * h + j] = Wr[:, 2 * h + f]
            w15[10, 12 + 2 * h + j] = br[2 * h + f]
        for f in range(2):
            w15[0:10, 4 + h] += at[h, f] * Ws[:, 2 * h + f]
            w15[10, 4 + h] += at[h, f] * bs[2 * h + f]
            w15[0:10, 10 + h] += at[h, f] * Wd[:, 2 * h + f]
            w15[10, 10 + h] += at[h, f] * bd[2 * h + f]
    inva2 = np.zeros(4, np.float32)
    for h in range(2):
        for j, f in enumerate(perm2[h]):
            inva2[2 * h + j] = 1.0 / aa[h, f]
    w1p2 = np.asarray(inputs['W1'], np.float32).copy()
    for h in range(2):
        for j, f in enumerate(perm2[h]):
            w1p2[2 + 2 * h + j] = np.asarray(inputs['W1'])[2 + 2 * h + f]
    return w15.astype(BF), inva2, tuple(npos2), perm2, w1p2


# ================================================================== weights
def d2_rowperm(w):
    out = np.zeros_like(w)
    for f in range(5):
        for h in range(2):
            out[2 * f + h] = w[5 * h + f]
    return out


def d2_colperm4(v):
    out = np.zeros_like(v)
    for f in range(2):
        for h in range(2):
            out[..., 2 * f + h] = v[..., 2 * h + f]
    return out


_cache = {}


def kernel(**inputs):
    x = np.asarray(inputs['x'], np.float32)
    src = np.asarray(inputs['src'], np.int32)
    dst = np.asarray(inputs['dst'], np.int32)
    n = x.shape[0]

    scheds, nst, Ls, offs, Lps, offs2 = build_schedule(dst, n)
    nt = scheds[0]['nt']
    nloc = scheds[0]['nloc']
    totc = int(offs[-1])
    totc2 = int(offs2[-1])
    ngl = -(-nt // 16)

    bd2, ml16, inva, npos, sa, perms = build_l1_weights(inputs)
    invat = np.tile(inva.reshape(1, 12), (P, 1)).astype(np.float32)
    w15, inva2, npos2, perm2, w1p2 = build_l2_weights(inputs)
    inva2t = np.tile(inva2.reshape(1, 4), (P, 1)).astype(np.float32)

    soff = np.concatenate([[0], np.cumsum([64 * Lp for Lp in Lps])]).astype(np.int64)
    s2c = int(soff[-1])
    ncl = -(-nloc // 512) * 512

    key = (n, len(src), nst, tuple(Lps), npos, sa, npos2)
    if key not in _cache:
        _cache[key] = (build_launch1_v2(nst, Lps, offs2, nt, ngl, npos, sa),
                       build_launch15(ncl),
                       build_launch2_v2(nst, Lps, soff, nt, npos2))
    nc1, nc15, nc2 = _cache[key]

    in1, core_meta = [], []
    for s in scheds:
        eo, pos_sorted, rank = edge_pos(s)
        rb2, col2 = edge_cols_v2(s, offs2, eo, pos_sorted, rank)
        core_meta.append((s, eo, pos_sorted, rank))
        x5e = pack_edges_v2(x[src[s['em']], :5], eo, rb2, col2, totc2)
        orig = s['order']
        valid = orig < nloc
        xl = np.zeros((nt * P, 5), np.float32)
        xl[valid] = x[s['core'] * nloc + orig[valid], :5]
        in1.append(dict(x5e=x5e, x5l=pack_xl16(xl, valid, nt, ngl),
                        bd2=bd2, ml16=ml16, inva=invat))
    r1 = run_bass_kernel_spmd(nc1, in1, core_ids=list(range(NCORE)))
    t1 = r1.exec_time_ns or 0

    # device h1 col order: [a1h0, a1h1, (h0: perm0), (h1: perm1)] -> std 5h+f
    hdef_g = np.zeros((n, 10), np.float32)
    hatt_percore = []
    for ci, s in enumerate(scheds):
        h1 = r1.results[ci]['h1o'].reshape(P, nt, 12).transpose(1, 0, 2).reshape(nt * P, 12)
        std = np.zeros((nt * P, 10), np.float32)
        for h in range(2):
            for j, f in enumerate(perms[h]):
                std[:, 5 * h + f] = h1[:, 2 + 5 * h + j]
        orig = s['order']
        valid = orig < nloc
        hdef_g[s['core'] * nloc + orig[valid]] = std[valid]
        hatt_percore.append(h1[:, 0:2])

    # launch 1.5: per-node d2 projections, nodes in global-id order
    in15 = []
    for c in range(NCORE):
        h1t = np.zeros((11, ncl), BF)
        h1t[0:10, :nloc] = hdef_g[c * nloc:(c + 1) * nloc].T.astype(BF)
        h1t[10, :nloc] = BF(1.0)
        in15.append(dict(h1t=h1t, w15=w15))
    r15 = run_bass_kernel_spmd(nc15, in15, core_ids=list(range(NCORE)))
    t15 = r15.exec_time_ns or 0
    fsall = np.zeros((16, n), np.float32)
    for c in range(NCORE):
        fsall[:, c * nloc:(c + 1) * nloc] = \
            r15.results[c]['p2'][:, :nloc].astype(np.float32)

    in2 = []
    for ci, (s, eo, pos_sorted, rank) in enumerate(core_meta):
        st_of = (pos_sorted // P) // B
        b_e = (pos_sorted // P) % B
        p_e = pos_sorted % P
        base = (soff[st_of] + rank * 64 + b_e * 8).astype(np.int64)
        srcg = src[s['em']][eo]
        pk2 = np.zeros((P, s2c), BF)
        fe = fsall[0:6, srcg].astype(BF)
        for c in range(6):
            pk2[p_e, base + c] = fe[c]
        pk2[p_e, base + 6] = BF(1.0)
        orig = s['order']
        valid = orig < nloc
        gid = s['core'] * nloc + orig
        v8 = np.zeros((nt * P, 8), np.float32)
        v8[valid, 0:6] = fsall[6:12, gid[valid]].T
        v4 = np.zeros((nt * P, 4), np.float32)
        v4[valid] = fsall[12:16, gid[valid]].T
        v14 = np.zeros((nt * P, 14), np.float32)
        v14[:, 0:2] = hatt_percore[ci]
        v14[valid, 6:14] = x[gid[valid], :]
        in2.append(dict(s2=pk2, ft2=pm(v8, nt).astype(BF),
                        fr2=pm(v4, nt).astype(BF), inva2=inva2t,
                        m14b=pm(v14, nt).astype(BF),
                        w1=w1p2.astype(BF),
                        w2=np.asarray(inputs['W2'], np.float32).astype(BF),
                        w3=np.asarray(inputs['W3'], np.float32).astype(BF),
                        w4=np.asarray(inputs['W4'], np.float32).astype(BF),
                        b1=np.asarray(inputs['b1'], np.float32),
                        b2=np.asarray(inputs['b2'], np.float32),
                        b3=np.asarray(inputs['b3'], np.float32),
                        b4=np.asarray(inputs['b4'], np.float32)))
    r2 = run_bass_kernel_spmd(nc2, in2, core_ids=list(range(NCORE)))
    t2 = r2.exec_time_ns or 0

    out = np.zeros((n, 1), np.float32)
    for ci, s in enumerate(scheds):
        y = r2.results[ci]['out'].reshape(nt * P)
        orig = s['order']
        valid = orig < nloc
        out[s['core'] * nloc + orig[valid], 0] = y[valid]
    kernel.last_exec_ns = t1 + t15 + t2
    kernel.last_t12 = (t1, t15, t2)
    kernel.last_results = (r1, r15, r2)
    return out
